# revision 9
# baseline (speedup 1.0000x reference)
"""Trainium2 distributed Bass kernel for the APGAT layer (gnn_message_passing).

v2 strategy (8 NeuronCores, 4 HBM pairs):
  - Nodes are sharded over cores (6272/core); cores (2k, 2k+1) share an HBM
    domain, so each pair holds a PAIR-LOCAL z table of 12544 rows in Shared
    DRAM (written with a rank-dependent dynamic-offset DMA; a tiny pair
    AllReduce acts as the readiness barrier).  12544 < 32768 so gather
    indices fit int16 with a SINGLE table - no A/B split.
  - Edges are assigned to the pair that owns their src node (gathers are
    always pair-local; no z AllGather at all), split evenly between the two
    cores of the pair.  Each core processes ~50k edges spanning ALL dst
    nodes; per-dst partial sums are combined at the end with two pipelined
    8-core ReduceScatters over the 10240 dst slots.
  - dst nodes are relabeled on the host (greedy, degree-balanced) into 80
    windows of 125 nodes so every (core, window) has <= C_WIN edges.
    Edge stream = 80 windows x C_WIN slots, padded with dummy (valid) idx
    rows that dstrel = -1 masks out of the one-hot.
  - Per 1792-edge chunk: SWDGE dma_gather of [z | s_src] rows (768 B),
    s_feat via PE (srl stationary), scores -> exp, msg = ex (x) z in the
    DVE 2x mode (d-major layout).  Per window: one-hot is_equal + PE
    segment-sum matmuls into PSUM, evicted bf16 to an SBUF accumulator.
  - Softmax max-subtraction is skipped (scores are O(7); validated vs the
    reference, gate is 2e-2).
"""

import sys

sys.path.insert(0, "/opt/trn_rl_repo")

import numpy as np
import ml_dtypes

import concourse.bass as bass
import concourse.bacc as bacc
import concourse.mybir as mybir
import concourse.tile as tile
from concourse.bass import ds
from concourse.tile import add_dep_helper
from concourse.bass_utils import run_bass_kernel_spmd

BF16 = ml_dtypes.bfloat16
F32 = np.float32
AF = mybir.ActivationFunctionType
ALU = mybir.AluOpType

N_SWDGE_Q = 4


class Cfg:
    def __init__(self, ncores=8, na=50000, np_=10000, e=400000, in_dim=512,
                 feat=128, h=8, d=32, c_win=672, chunk=896, single_packet=True):
        self.NCORES = ncores
        self.NPAIRS = ncores // 2
        self.NA = na
        self.NP = np_
        self.E = e
        self.IN_DIM = in_dim
        self.KC = in_dim // 128
        self.FEAT = feat
        self.H = h
        self.D = d
        self.HD = h * d                    # 256
        self.HDE = self.HD + h             # 264: [z | s_src]
        self.RW = 384                      # bf16 row width -> 768 B rows
        nsh = -(-na // ncores)
        self.NSH = -(-nsh // 128) * 128    # nodes per core (6272)
        self.NA_PAD = self.NSH * ncores
        self.NT = self.NSH // 128          # node tiles per core (49)
        self.PAIR_ROWS = 2 * self.NSH      # 12544 (< 32768: int16 idx)
        assert self.PAIR_ROWS <= 32768
        # dst windows: 80 windows x 125 nodes (128 slots each)
        self.NWIN = 80
        self.NODES_PER_WIN = np_ // self.NWIN    # 125
        self.SLOT_ROWS = self.NWIN * 128         # 10240 dst slots
        self.WIN_PER_CORE = self.NWIN // ncores  # 10
        # per (core, window) edge capacity
        assert c_win % 16 == 0
        self.C_WIN = c_win
        self.CHUNK = chunk                 # gather chunk (multiple of 128)
        assert chunk % 128 == 0
        self.SP = single_packet
        slots = self.NWIN * c_win
        self.NCHUNK = -(-slots // chunk)
        self.SLOTS = self.NCHUNK * chunk   # padded stream length
        self.TPC = chunk // 128            # tiles per chunk
        self.NTILES = self.SLOTS // 128
        # per-window tile spans (static given C_WIN)
        self.win_t0 = [(c_win * w) // 128 for w in range(self.NWIN)]
        self.win_t1 = [-(-(c_win * (w + 1)) // 128) for w in range(self.NWIN)]
        self.win_blk0 = np.cumsum([0] + [t1 - t0 for t0, t1 in
                                         zip(self.win_t0, self.win_t1)])
        self.NBLK = int(self.win_blk0[-1])


def build_graph(cfg: Cfg):
    nc = bacc.Bacc("TRN2", target_bir_lowering=False, debug=False,
                   num_devices=cfg.NCORES, num_swdge_queues=N_SWDGE_Q)
    bf = mybir.dt.bfloat16
    f32 = mybir.dt.float32
    i16 = mybir.dt.int16

    # ---- kernel I/O ----
    hT = nc.dram_tensor("hT", [128, cfg.KC, cfg.NSH], bf, kind="ExternalInput")
    WfcT = nc.dram_tensor("WfcT", [128, cfg.KC, cfg.HDE], bf, kind="ExternalInput")
    WfeT = nc.dram_tensor("WfeT", [cfg.FEAT, cfg.H], bf, kind="ExternalInput")
    IotaM = nc.dram_tensor("IotaM", [128, 128], bf, kind="ExternalInput")
    srlT = nc.dram_tensor("srlT", [cfg.FEAT, cfg.SLOTS], bf, kind="ExternalInput")
    dstrel = nc.dram_tensor("dstrel", [128, cfg.NBLK], bf, kind="ExternalInput")
    idxT = nc.dram_tensor("idxT", [128, cfg.SLOTS // 16], i16, kind="ExternalInput")
    out_ext = nc.dram_tensor("out", [2 * cfg.WIN_PER_CORE // 2 * 128, cfg.HD],
                             f32, kind="ExternalOutput")   # [1280, 256]

    pair_groups = [[2 * p, 2 * p + 1] for p in range(cfg.NPAIRS)]
    all_group = [list(range(cfg.NCORES))]
    WPH = cfg.NWIN // 2                    # windows per RS half (40)
    SPH = WPH * 128                        # acc rows per half (5120)
    WPC_H = WPH // cfg.NCORES              # windows per core per half (5)

    with tile.TileContext(nc) as tc:
        with (
            tc.tile_pool(name="dram", bufs=1, space="DRAM") as dram,
            tc.tile_pool(name="consts", bufs=1) as consts,
            tc.tile_pool(name="psum_sf", bufs=2, space="PSUM") as psum_sf,
            tc.tile_pool(name="psum_acc", bufs=2, space="PSUM") as psum_acc,
            tc.tile_pool(name="srl", bufs=3) as srl_pool,
            tc.tile_pool(name="zg", bufs=4) as zg_pool,
            tc.tile_pool(name="msg", bufs=4) as msg_pool,
            tc.tile_pool(name="oh", bufs=3) as oh_pool,
            tc.tile_pool(name="small", bufs=3) as small,
            tc.tile_pool(name="accsb", bufs=1) as accsb,
            tc.tile_pool(name="fin", bufs=2) as fin,
        ):
            # ---- shared/DRAM scratch ----
            z_pair = dram.tile([cfg.PAIR_ROWS, cfg.RW], bf, addr_space="Shared")
            bar_in = dram.tile([1, 16], bf)
            bar_out = dram.tile([1, 16], bf)
            acc_dram = [dram.tile([SPH, cfg.HDE], bf, name=f"accd{i}")
                        for i in range(2)]
            rs_out = [dram.tile([SPH // cfg.NCORES, cfg.HDE], bf, name=f"rs{i}")
                      for i in range(2)]

            # ---- constants ----
            wfe_sb = consts.tile([cfg.FEAT, cfg.H], bf)
            nc.sync.dma_start(wfe_sb[:], WfeT[:])
            iota_sb = consts.tile([128, 128], bf)
            nc.sync.dma_start(iota_sb[:], IotaM[:])
            idx_sb = consts.tile([128, cfg.SLOTS // 16], i16)
            nc.sync.dma_start(idx_sb[:], idxT[:])
            dst_sb = consts.tile([128, cfg.NBLK], bf)
            nc.sync.dma_start(dst_sb[:], dstrel[:])

            # rank within the pair (0/1) for the z-table write offset
            rank1 = nc.gpsimd.partition_id() % 2
            row_off = rank1 * cfg.NSH

            # ---- phase A: z = [h @ Wfc | s_src] -> pair-shared table ----
            with (
                tc.tile_pool(name="zph_h", bufs=2) as zph_h,
                tc.tile_pool(name="zph_w", bufs=1) as zph_w,
                tc.tile_pool(name="zph_s", bufs=1) as zph_s,
                tc.tile_pool(name="psum_z", bufs=4, space="PSUM") as psum_z,
            ):
                wfc_sb = zph_w.tile([128, cfg.KC, cfg.HDE], bf)
                nc.sync.dma_start(wfc_sb[:], WfcT[:])
                zstage = zph_s.tile([128, cfg.NT, cfg.RW], bf)
                bounds = [0, 13, 25, 37, cfg.NT]
                for t0, t1 in zip(bounds[:-1], bounds[1:]):
                    nt = t1 - t0
                    hT_sb = zph_h.tile([128, cfg.KC, 13 * 128], bf,
                                       name=f"hT{t0}", tag="hT")
                    nc.sync.dma_start(hT_sb[:, :, 0:nt * 128],
                                      hT[:, :, t0 * 128:t1 * 128])
                    for i in range(nt):
                        pz = psum_z.tile([128, cfg.HDE], f32,
                                         name=f"pz{t0 + i}", tag="pz")
                        for c in range(cfg.KC):
                            nc.tensor.matmul(
                                pz[:],
                                hT_sb[:, c, i * 128:(i + 1) * 128],
                                wfc_sb[:, c, :],
                                start=(c == 0), stop=(c == cfg.KC - 1),
                            )
                        nc.scalar.copy(zstage[:, t0 + i, 0:cfg.HDE], pz[:])
                z_write = nc.gpsimd.dma_start(
                    z_pair[:][ds(row_off, cfg.NSH), :]
                        .rearrange("(t p) r -> p t r", p=128),
                    zstage[:])

            # barrier: pair AllReduce; completes only after both pair cores
            # have finished their z-table writes
            nc.sync.dma_start(bar_in[:], iota_sb[0:1, 0:16])
            bar = nc.gpsimd.collective_compute(
                "AllReduce", ALU.add, ins=[bar_in[:].opt()],
                outs=[bar_out[:].opt()], replica_groups=pair_groups)
            add_dep_helper(bar.ins, z_write.ins, reason="barrier after z write")

            # ---- phase B ----
            # windows grouped by the chunk that completes them
            win_by_chunk = {}
            for w in range(cfg.NWIN):
                lc = (cfg.win_t1[w] - 1) // cfg.TPC
                win_by_chunk.setdefault(lc, []).append(w)

            msg_tiles = {}    # global tile idx -> (msg tile, local idx)
            acc_sb = [accsb.tile([128, WPH, cfg.HDE], bf, name=f"acc{i}")
                      for i in range(2)]

            def emit_window(w):
                t0, t1 = cfg.win_t0[w], cfg.win_t1[w]
                ntw = t1 - t0
                blk0 = int(cfg.win_blk0[w])
                oh = oh_pool.tile([128, ntw, 128], bf, name=f"oh{w}", tag="oh")
                nc.vector.tensor_tensor(
                    oh[:],
                    iota_sb[:].unsqueeze(1).broadcast_to([128, ntw, 128]),
                    dst_sb[:, blk0:blk0 + ntw]
                        .unsqueeze(2).broadcast_to([128, ntw, 128]),
                    ALU.is_equal,
                )
                pacc = psum_acc.tile([128, cfg.HDE], f32,
                                     name=f"pacc{w}", tag="pacc")
                for j, t in enumerate(range(t0, t1)):
                    mt, li = msg_tiles[t]
                    nc.tensor.matmul(
                        pacc[:],
                        oh[:, j, :],
                        mt[:, li, :],
                        start=(j == 0), stop=(j == ntw - 1),
                    )
                half, wl = w // WPH, w % WPH
                nc.scalar.copy(acc_sb[half][:, wl, :], pacc[:])

            rs_insts = []

            def emit_rs(half):
                nc.sync.dma_start(
                    acc_dram[half][:]
                        .rearrange("(w p) c -> p w c", p=128),
                    acc_sb[half][:])
                rs = nc.gpsimd.collective_compute(
                    "ReduceScatter", ALU.add,
                    ins=[acc_dram[half][:].opt()],
                    outs=[rs_out[half][:].opt()],
                    replica_groups=all_group)
                rs_insts.append(rs)

            for c in range(cfg.NCHUNK):
                zg = zg_pool.tile([128, cfg.TPC, cfg.RW], bf,
                                  name=f"zg{c}", tag="zg")
                g = nc.gpsimd.dma_gather(
                    zg[:], z_pair[:],
                    idx_sb[:, c * (cfg.CHUNK // 16):(c + 1) * (cfg.CHUNK // 16)],
                    cfg.CHUNK, cfg.CHUNK, cfg.RW,
                    single_packet=cfg.SP,
                    queue_num=c % N_SWDGE_Q,
                )
                add_dep_helper(g.ins, bar.ins, reason="gather after barrier")

                srl_sb = srl_pool.tile([cfg.FEAT, cfg.CHUNK], bf,
                                       name=f"srl{c}", tag="srl")
                nc.scalar.dma_start(
                    srl_sb[:], srlT[:, c * cfg.CHUNK:(c + 1) * cfg.CHUNK])

                psf = psum_sf.tile([128, cfg.TPC, cfg.H], f32,
                                   name=f"psf{c}", tag="psf")
                for t in range(cfg.TPC):
                    nc.tensor.matmul(
                        psf[:, t, :],
                        srl_sb[:, t * 128:(t + 1) * 128],
                        wfe_sb[:],
                        start=True, stop=True,
                    )

                # scores: s = s_src + s_feat -> lrelu -> exp
                n_sf = cfg.TPC * cfg.H
                ss = small.tile([128, cfg.TPC, cfg.H], f32,
                                name=f"ss{c}", tag="ss")
                nc.scalar.copy(ss[:], zg[:, :, cfg.HD:cfg.HDE])
                sall = small.tile([128, n_sf], f32, name=f"sall{c}", tag="sall")
                nc.vector.tensor_tensor(
                    sall[:].rearrange("p (t h) -> p t h", h=cfg.H),
                    ss[:], psf[:], ALU.add)
                slr = small.tile([128, n_sf], f32, name=f"slr{c}", tag="slr")
                nc.vector.scalar_tensor_tensor(
                    slr[:], sall[:], 0.01, sall[:], ALU.mult, ALU.max)
                exf = small.tile([128, n_sf], f32, name=f"exf{c}", tag="exf")
                nc.scalar.activation(exf[:], slr[:], AF.Exp)

                msg = msg_pool.tile([128, cfg.TPC, cfg.HDE], bf,
                                    name=f"msg{c}", tag="msg")
                nc.scalar.copy(
                    msg[:, :, cfg.HD:],
                    exf[:].rearrange("p (t h) -> p t h", h=cfg.H))
                nc.vector.tensor_tensor(
                    msg[:, :, 0:cfg.HD].rearrange("p t (d h) -> p t d h", h=cfg.H),
                    zg[:, :, 0:cfg.HD].rearrange("p t (d h) -> p t d h", h=cfg.H),
                    msg[:, :, cfg.HD:].unsqueeze(2)
                        .broadcast_to([128, cfg.TPC, cfg.D, cfg.H]),
                    ALU.mult,
                )
                for t in range(cfg.TPC):
                    msg_tiles[c * cfg.TPC + t] = (msg, t)

                for w in win_by_chunk.get(c, []):
                    emit_window(w)
                    if w == WPH - 1:
                        emit_rs(0)
                if c == cfg.NCHUNK - 1:
                    emit_rs(1)

            # ---- finalize: out = msg_tot / max(den, eps) per RS shard ----
            for half in range(2):
                tot = fin.tile([128, WPC_H, cfg.HDE], bf,
                               name=f"tot{half}", tag="tot")
                nc.sync.dma_start(
                    tot[:],
                    rs_out[half][:].rearrange("(w p) c -> p w c", p=128))
                den = fin.tile([128, WPC_H, cfg.H], f32,
                               name=f"den{half}", tag="den")
                nc.scalar.activation(den[:], tot[:, :, cfg.HD:cfg.HDE],
                                     AF.Copy, bias=1e-9)
                rec = fin.tile([128, WPC_H, cfg.H], f32,
                               name=f"rec{half}", tag="rec")
                nc.vector.reciprocal(rec[:], den[:])
                ow = fin.tile([128, WPC_H, cfg.HD], f32,
                              name=f"ow{half}", tag="ow")
                nc.vector.tensor_tensor(
                    ow[:].rearrange("p w (d h) -> p w d h", h=cfg.H),
                    tot[:, :, 0:cfg.HD].rearrange("p w (d h) -> p w d h", h=cfg.H),
                    rec[:].unsqueeze(2)
                        .broadcast_to([128, WPC_H, cfg.D, cfg.H]),
                    ALU.mult,
                )
                nc.sync.dma_start(
                    out_ext[half * WPC_H * 128:(half + 1) * WPC_H * 128, :]
                        .rearrange("(w p) c -> p w c", p=128),
                    ow[:])

    nc.compile()
    return nc


# --------------------------------------------------------------------------
# host-side preprocessing
# --------------------------------------------------------------------------

def _greedy_windows(cfg: Cfg, dst, pair_of_edge):
    """Assign dst nodes to 80 windows of 125, balancing per-pair edge load.
    Returns slot_of_dst [NP] (window*128 + position)."""
    NW = cfg.NWIN
    # per (dst, pair) degree
    deg = np.zeros((cfg.NP, cfg.NPAIRS), np.int64)
    np.add.at(deg, (dst, pair_of_edge), 1)
    tot = deg.sum(1)
    order = np.argsort(-tot, kind="stable")
    load = np.zeros((NW, cfg.NPAIRS), np.int64)
    count = np.zeros(NW, np.int64)
    win_of = np.empty(cfg.NP, np.int64)
    pos_of = np.empty(cfg.NP, np.int64)
    for d in order:
        cand = load + deg[d][None, :]
        score = cand.max(1) * 1000 + cand.sum(1)
        score[count >= cfg.NODES_PER_WIN] = np.iinfo(np.int64).max
        w = int(np.argmin(score))
        win_of[d] = w
        pos_of[d] = count[w]
        count[w] += 1
        load[w] += deg[d]
    assert (count == cfg.NODES_PER_WIN).all()
    return win_of * 128 + pos_of


def host_prep(cfg: Cfg, h, srl_emb, src, dst, W_fc, W_feat, W_attn):
    H, D = cfg.H, cfg.D

    a = np.asarray(W_attn, F32)[0]
    a_src, a_feat = a[:D], a[2 * D:3 * D]
    W_fc = np.asarray(W_fc, F32)
    Wf_eff = (np.asarray(W_feat, F32).reshape(H, D, cfg.FEAT)
              * a_feat[None, :, None]).sum(1)
    Wz_eff = (W_fc.reshape(H, D, cfg.IN_DIM) * a_src[None, :, None]).sum(1)

    # d-major column order for z: col j <-> (h=j%8, d=j//8)
    perm = np.array([(j % H) * D + j // H for j in range(cfg.HD)], np.int64)
    Wfull = np.concatenate([W_fc.T[:, perm], Wz_eff.T], axis=1)  # [IN_DIM, HDE]
    WfcT_r = np.ascontiguousarray(
        Wfull.reshape(cfg.KC, 128, cfg.HDE).transpose(1, 0, 2)).astype(BF16)
    WfeT_r = np.ascontiguousarray(Wf_eff.T).astype(BF16)
    IotaM = np.tile(np.arange(128, dtype=F32)[None, :], (128, 1)).astype(BF16)

    h_bf = np.zeros((cfg.NA_PAD, cfg.IN_DIM), BF16)
    h_bf[:cfg.NA] = np.asarray(h, F32).astype(BF16)
    srl_bf = np.asarray(srl_emb, F32).astype(BF16)

    src = np.asarray(src, np.int64)
    dst = np.asarray(dst, np.int64)
    pair_of_edge = src // cfg.PAIR_ROWS          # src in padded node space
    slot_of_dst = _greedy_windows(cfg, dst, pair_of_edge)
    win_of_edge = slot_of_dst[dst] // 128

    # core assignment: within (pair, window), alternate between pair cores
    order = np.lexsort((src, win_of_edge, pair_of_edge))
    e_s = order
    pair_s = pair_of_edge[e_s]
    win_s = win_of_edge[e_s]
    key = pair_s * cfg.NWIN + win_s
    # rank within each (pair, window) group
    grp_start = np.r_[True, key[1:] != key[:-1]]
    gidx = np.arange(len(e_s)) - np.maximum.accumulate(
        np.where(grp_start, np.arange(len(e_s)), 0))
    core_s = pair_s * 2 + (gidx % 2)

    in_maps = []
    for c in range(cfg.NCORES):
        sel = core_s == c
        e_c = e_s[sel]                     # sorted by (window, src)
        win_c = win_s[sel]
        cnt = np.bincount(win_c, minlength=cfg.NWIN)
        assert cnt.max() <= cfg.C_WIN, f"C_WIN too small: {cnt.max()}"

        idx = np.zeros(cfg.SLOTS, np.int16)
        dstrel_v = np.full(cfg.SLOTS, -1.0, F32)
        srl_rows = np.zeros((cfg.SLOTS, cfg.FEAT), BF16)
        pos = win_c * cfg.C_WIN + (
            np.arange(len(e_c)) - np.r_[0, np.cumsum(cnt)][win_c])
        row = (src[e_c] - (c // 2) * cfg.PAIR_ROWS).astype(np.int16)
        idx[pos] = row
        dstrel_v[pos] = (slot_of_dst[dst[e_c]] - win_c * 128).astype(F32)
        srl_rows[pos] = srl_bf[e_c]
        # dummy pads: repeat a valid row (idx stays 0 where no edge before;
        # fill window pads with the window's first real row for locality)
        for w in range(cfg.NWIN):
            if cnt[w] < cfg.C_WIN:
                fill = row[np.searchsorted(win_c, w)] if cnt[w] > 0 else 0
                idx[w * cfg.C_WIN + cnt[w]:(w + 1) * cfg.C_WIN] = fill

        srlT_c = np.ascontiguousarray(srl_rows.T)

        # dstrel blocks: per (window, tile-in-window) columns
        dstrel_blk = np.full((128, cfg.NBLK), -1.0, F32)
        slots_v = dstrel_v.reshape(cfg.NTILES, 128).T   # [128, tile]
        for w in range(cfg.NWIN):
            t0, t1 = cfg.win_t0[w], cfg.win_t1[w]
            b0 = int(cfg.win_blk0[w])
            base_shift = np.zeros(t1 - t0, F32)
            # dstrel_v holds slot - win*128 for the edge's own window; for a
            # straddle tile the neighbor window's edges carry their own
            # offset.  Rebase everything to window w:
            for j, t in enumerate(range(t0, t1)):
                col = slots_v[:, t].copy()
                # which window does each slot position belong to?
                slot_ids = t * 128 + np.arange(128)
                w_of_slot = slot_ids // cfg.C_WIN
                valid = col >= 0
                rb = col + (w_of_slot - w) * 128.0
                rb[~valid] = -1.0
                dstrel_blk[:, b0 + j] = rb
        dstrel_c = dstrel_blk.astype(BF16)

        def wrap_idx(arr):  # [SLOTS] -> [128, SLOTS//16]
            wr = arr.reshape(cfg.SLOTS // 16, 16).T
            return np.ascontiguousarray(np.tile(wr, (8, 1)))

        hsl = h_bf[c * cfg.NSH:(c + 1) * cfg.NSH]
        hT_c = np.ascontiguousarray(
            hsl.T.reshape(cfg.KC, 128, cfg.NSH).transpose(1, 0, 2))

        in_maps.append({
            "hT": hT_c,
            "WfcT": WfcT_r,
            "WfeT": WfeT_r,
            "IotaM": IotaM,
            "srlT": srlT_c,
            "dstrel": dstrel_c,
            "idxT": wrap_idx(idx),
        })
    return in_maps, slot_of_dst


def required_c_win(cfg: Cfg, src, dst):
    src = np.asarray(src, np.int64)
    dst = np.asarray(dst, np.int64)
    pair_of_edge = src // cfg.PAIR_ROWS
    slot_of_dst = _greedy_windows(cfg, dst, pair_of_edge)
    win_of_edge = slot_of_dst[dst] // 128
    # worst core count: ceil(pair-window count / 2)
    key = pair_of_edge * cfg.NWIN + win_of_edge
    counts = np.bincount(key, minlength=cfg.NPAIRS * cfg.NWIN)
    need = int(-(-counts.max() // 2))
    return -(-need // 16) * 16


# --------------------------------------------------------------------------
# entry point
# --------------------------------------------------------------------------

_CACHE = {}


def _get_graph(cfg: Cfg):
    key = (cfg.NCORES, cfg.NA_PAD, cfg.NP, cfg.C_WIN, cfg.CHUNK, cfg.SP)
    if key not in _CACHE:
        _CACHE[key] = build_graph(cfg)
    return _CACHE[key]


def kernel(h, srl_emb, src, dst, W_fc, W_feat, W_attn, _trace=False,
           _tmpdir=None):
    src = np.asarray(src)
    dst = np.asarray(dst)
    cfg = Cfg()
    need = required_c_win(cfg, src, dst)
    if need > cfg.C_WIN:
        cfg = Cfg(c_win=need)
    nc = _get_graph(cfg)
    in_maps, slot_of_dst = host_prep(
        cfg, np.asarray(h), np.asarray(srl_emb), src, dst,
        np.asarray(W_fc), np.asarray(W_feat), np.asarray(W_attn))
    res = run_bass_kernel_spmd(nc, in_maps, core_ids=list(range(cfg.NCORES)),
                               trace=_trace, tmpdir=_tmpdir)
    # reassemble: core c, half h rows cover windows h*40 + 5c + [0..5)
    out_slots = np.empty((cfg.SLOT_ROWS, cfg.H, cfg.D), F32)
    WPH = cfg.NWIN // 2
    WPC_H = WPH // cfg.NCORES
    for c in range(cfg.NCORES):
        shard = np.asarray(res.results[c]["out"], F32)  # [1280, 256] d-major
        shard = shard.reshape(2, WPC_H * 128, cfg.D, cfg.H).transpose(0, 1, 3, 2)
        for half in range(2):
            w0 = half * WPH + c * WPC_H
            out_slots[w0 * 128:(w0 + WPC_H) * 128] = shard[half]
    out = out_slots[slot_of_dst]
    if _trace:
        kernel._last_results = res
    return out


# revision 33
# speedup vs baseline: 1.2425x; 1.2425x over previous
"""Trainium2 distributed Bass kernel for the APGAT layer (gnn_message_passing).

v2 strategy (8 NeuronCores, 4 HBM pairs):
  - Nodes are sharded over cores (6272/core); cores (2k, 2k+1) share an HBM
    domain, so each pair holds a PAIR-LOCAL z table of 12544 rows in Shared
    DRAM (written with a rank-dependent dynamic-offset DMA; a tiny pair
    AllReduce acts as the readiness barrier).  12544 < 32768 so gather
    indices fit int16 with a SINGLE table - no A/B split.
  - Edges are assigned to the pair that owns their src node (gathers are
    always pair-local; no z AllGather at all), split evenly between the two
    cores of the pair.  Each core processes ~50k edges spanning ALL dst
    nodes; per-dst partial sums are combined at the end with two pipelined
    8-core ReduceScatters over the 10240 dst slots.
  - dst nodes are relabeled on the host (greedy, degree-balanced) into 80
    windows of 125 nodes so every (core, window) has <= C_WIN edges.
    Edge stream = 80 windows x C_WIN slots, padded with dummy (valid) idx
    rows that dstrel = -1 masks out of the one-hot.
  - Per 1792-edge chunk: SWDGE dma_gather of [z | s_src] rows (768 B),
    s_feat via PE (srl stationary), scores -> exp, msg = ex (x) z in the
    DVE 2x mode (d-major layout).  Per window: one-hot is_equal + PE
    segment-sum matmuls into PSUM, evicted bf16 to an SBUF accumulator.
  - Softmax max-subtraction is skipped (scores are O(7); validated vs the
    reference, gate is 2e-2).
"""

import sys

sys.path.insert(0, "/opt/trn_rl_repo")

import numpy as np
import ml_dtypes

import concourse.bass as bass
import concourse.bacc as bacc
import concourse.mybir as mybir
import concourse.tile as tile
from concourse.bass import ds
from concourse.tile import add_dep_helper
from concourse.bass_utils import run_bass_kernel_spmd

BF16 = ml_dtypes.bfloat16
F8 = ml_dtypes.float8_e4m3fn
F32 = np.float32
AF = mybir.ActivationFunctionType
ALU = mybir.AluOpType

N_SWDGE_Q = 8


class Cfg:
    def __init__(self, ncores=8, na=50000, np_=10000, e=400000, in_dim=512,
                 feat=128, h=8, d=32, c_win=640, chunk=896, single_packet=True):
        self.NCORES = ncores
        self.NPAIRS = ncores // 2
        self.NA = na
        self.NP = np_
        self.E = e
        self.IN_DIM = in_dim
        self.KC = in_dim // 128
        self.FEAT = feat
        self.H = h
        self.D = d
        self.HD = h * d                    # 256
        self.HDE = self.HD + h             # 264: [z | s_src]
        self.RW = 384                      # bf16 row width -> 768 B rows
        nsh = -(-na // ncores)
        self.NSH = -(-nsh // 128) * 128    # nodes per core (6272)
        self.NA_PAD = self.NSH * ncores
        self.NT = self.NSH // 128          # node tiles per core (49)
        self.PAIR_ROWS = 2 * self.NSH      # 12544 (< 32768: int16 idx)
        assert self.PAIR_ROWS <= 32768
        # dst windows: 80 windows x 125 nodes (128 slots each)
        self.NWIN = 80
        self.NODES_PER_WIN = np_ // self.NWIN    # 125
        self.SLOT_ROWS = self.NWIN * 128         # 10240 dst slots
        self.WIN_PER_CORE = self.NWIN // ncores  # 10
        # per (core, window) edge capacity
        assert c_win % 16 == 0
        self.C_WIN = c_win
        self.CHUNK = chunk                 # gather chunk (multiple of 128)
        assert chunk % 128 == 0
        self.SP = single_packet
        slots = self.NWIN * c_win
        self.NCHUNK = -(-slots // chunk)
        self.SLOTS = self.NCHUNK * chunk   # padded stream length
        self.TPC = chunk // 128            # tiles per chunk
        self.NTILES = self.SLOTS // 128
        # per-window tile spans (static given C_WIN)
        self.win_t0 = [(c_win * w) // 128 for w in range(self.NWIN)]
        self.win_t1 = [-(-(c_win * (w + 1)) // 128) for w in range(self.NWIN)]
        self.win_blk0 = np.cumsum([0] + [t1 - t0 for t0, t1 in
                                         zip(self.win_t0, self.win_t1)])
        self.NBLK = int(self.win_blk0[-1])


def build_graph(cfg: Cfg):
    nc = bacc.Bacc("TRN2", target_bir_lowering=False, debug=False,
                   num_devices=cfg.NCORES, num_swdge_queues=N_SWDGE_Q)
    bf = mybir.dt.bfloat16
    f8 = mybir.dt.float8e4
    f32 = mybir.dt.float32
    i16 = mybir.dt.int16
    u8 = mybir.dt.uint8

    # ---- kernel I/O ----
    hT = nc.dram_tensor("hT", [128, cfg.KC, cfg.NSH], bf, kind="ExternalInput")
    WfcT = nc.dram_tensor("WfcT", [128, cfg.KC, cfg.HDE], bf, kind="ExternalInput")
    WfeT = nc.dram_tensor("WfeT", [cfg.FEAT, cfg.H], bf, kind="ExternalInput")
    IotaM = nc.dram_tensor("IotaM", [128, 128], bf, kind="ExternalInput")
    srlT = nc.dram_tensor("srlT", [cfg.FEAT, cfg.SLOTS], bf, kind="ExternalInput")
    dstrel = nc.dram_tensor("dstrel", [128, cfg.NBLK], bf, kind="ExternalInput")
    idxT = nc.dram_tensor("idxT", [128, cfg.SLOTS // 16], i16, kind="ExternalInput")
    out_ext = nc.dram_tensor("out", [2 * cfg.WIN_PER_CORE // 2 * 128, cfg.HD],
                             f32, kind="ExternalOutput")   # [1280, 256]

    pair_groups = [[2 * p, 2 * p + 1] for p in range(cfg.NPAIRS)]
    all_group = [list(range(cfg.NCORES))]
    NSEG = 2                               # pipelined ReduceScatter halves
    WPS = cfg.NWIN // NSEG                 # windows per segment (16)
    SPS = WPS * 128                        # acc rows per segment (2048)
    SHR = SPS // cfg.NCORES                # shard rows per core (256)
    SHT = SHR // 128                       # shard row tiles (2)

    with tile.TileContext(nc) as tc:
        with (
            tc.tile_pool(name="dram", bufs=1, space="DRAM") as dram,
            tc.tile_pool(name="consts", bufs=1) as consts,
            tc.tile_pool(name="psum_sf", bufs=2, space="PSUM") as psum_sf,
            tc.tile_pool(name="psum_acc", bufs=2, space="PSUM") as psum_acc,
            tc.tile_pool(name="srl", bufs=4) as srl_pool,
            tc.tile_pool(name="zg", bufs=7) as zg_pool,
            tc.tile_pool(name="msg", bufs=6) as msg_pool,
            tc.tile_pool(name="oh", bufs=3) as oh_pool,
            tc.tile_pool(name="small", bufs=3) as small,
            tc.tile_pool(name="accsb", bufs=1) as accsb,
            tc.tile_pool(name="fin", bufs=2) as fin,
        ):
            # ---- shared/DRAM scratch ----
            z_pair = dram.tile([cfg.PAIR_ROWS, cfg.RW], bf, addr_space="Shared")
            bar_in = dram.tile([1, 16], bf)
            bar_out = dram.tile([1, 16], bf)
            acc_dram = [dram.tile([SPS, cfg.HDE], bf, name=f"accd{i}")
                        for i in range(NSEG)]
            rs_out = [dram.tile([SHR, cfg.HDE], bf, name=f"rs{i}")
                      for i in range(NSEG)]

            # ---- constants ----
            wfe_sb = consts.tile([cfg.FEAT, cfg.H], bf)
            nc.sync.dma_start(wfe_sb[:], WfeT[:])
            iota_sb = consts.tile([128, 128], bf)
            nc.sync.dma_start(iota_sb[:], IotaM[:])
            idx_sb = consts.tile([128, cfg.SLOTS // 16], i16)
            nc.sync.dma_start(idx_sb[:], idxT[:])
            dst_sb = consts.tile([128, cfg.NBLK], bf)
            nc.sync.dma_start(dst_sb[:], dstrel[:])

            # rank within the pair (0/1) for the z-table write offset
            rank1 = nc.sync.partition_id() % 2
            row_off = rank1 * cfg.NSH

            # ---- phase A: z = [h @ Wfc | s_src] -> pair-shared table ----
            with (
                tc.tile_pool(name="zph_h", bufs=2) as zph_h,
                tc.tile_pool(name="zph_w", bufs=1) as zph_w,
                tc.tile_pool(name="zph_s", bufs=1) as zph_s,
                tc.tile_pool(name="psum_z", bufs=4, space="PSUM") as psum_z,
            ):
                wfc_sb = zph_w.tile([128, cfg.KC, cfg.HDE], bf)
                nc.sync.dma_start(wfc_sb[:], WfcT[:])
                zstage = zph_s.tile([128, cfg.NT, cfg.HDE], bf)
                bounds = [0, 13, 25, 37, cfg.NT]
                for t0, t1 in zip(bounds[:-1], bounds[1:]):
                    nt = t1 - t0
                    hT_sb = zph_h.tile([128, cfg.KC, 13 * 128], bf,
                                       name=f"hT{t0}", tag="hT")
                    nc.sync.dma_start(hT_sb[:, :, 0:nt * 128],
                                      hT[:, :, t0 * 128:t1 * 128])
                    for i in range(nt):
                        pz = psum_z.tile([128, cfg.HDE], f32,
                                         name=f"pz{t0 + i}", tag="pz")
                        for c in range(cfg.KC):
                            nc.tensor.matmul(
                                pz[:],
                                hT_sb[:, c, i * 128:(i + 1) * 128],
                                wfc_sb[:, c, :],
                                start=(c == 0), stop=(c == cfg.KC - 1),
                            )
                        nc.scalar.copy(zstage[:, t0 + i, :], pz[:])
                z_write = nc.sync.dma_start(
                    z_pair[:][ds(row_off, cfg.NSH), 0:cfg.HDE]
                        .rearrange("(t p) r -> p t r", p=128),
                    zstage[:])

            # barrier: pair AllReduce; completes only after both pair cores
            # have finished their z-table writes
            nc.sync.dma_start(bar_in[:], iota_sb[0:1, 0:16])
            bar = nc.gpsimd.collective_compute(
                "AllReduce", ALU.add, ins=[bar_in[:].opt()],
                outs=[bar_out[:].opt()], replica_groups=pair_groups)
            add_dep_helper(bar.ins, z_write.ins, reason="barrier after z write")

            # ---- phase B ----
            # windows grouped by the chunk that completes them
            win_by_chunk = {}
            for w in range(cfg.NWIN):
                lc = (cfg.win_t1[w] - 1) // cfg.TPC
                win_by_chunk.setdefault(lc, []).append(w)

            msg_tiles = {}    # global tile idx -> (msg tile, local idx)
            evicts = []
            rs_pending = []
            acc_sb = [accsb.tile([128, WPS, cfg.HDE], bf, name=f"acc{i}")
                      for i in range(NSEG)]

            # prefetch: first srl chunks + one-hot blocks run during phase A
            srl_pre = {}
            for c in range(3):
                srl_sb = srl_pool.tile([cfg.FEAT, cfg.CHUNK], bf,
                                       name=f"srl{c}", tag="srl")
                nc.scalar.dma_start(
                    srl_sb[:], srlT[:, c * cfg.CHUNK:(c + 1) * cfg.CHUNK])
                srl_pre[c] = srl_sb

            oh_pre = {}

            def build_oh(w):
                t0, t1 = cfg.win_t0[w], cfg.win_t1[w]
                ntw = t1 - t0
                blk0 = int(cfg.win_blk0[w])
                oh = oh_pool.tile([128, ntw, 128], bf, name=f"oh{w}", tag="oh")
                nc.vector.tensor_tensor(
                    oh[:],
                    iota_sb[:].unsqueeze(1).broadcast_to([128, ntw, 128]),
                    dst_sb[:, blk0:blk0 + ntw]
                        .unsqueeze(2).broadcast_to([128, ntw, 128]),
                    ALU.is_equal,
                )
                return oh

            for w in range(3):
                oh_pre[w] = build_oh(w)

            def emit_window(w):
                t0, t1 = cfg.win_t0[w], cfg.win_t1[w]
                ntw = t1 - t0
                oh = oh_pre.pop(w, None)
                if oh is None:
                    oh = build_oh(w)
                pacc = psum_acc.tile([128, cfg.HDE], f32,
                                     name=f"pacc{w}", tag="pacc")
                for j, t in enumerate(range(t0, t1)):
                    mt, li = msg_tiles[t]
                    nc.tensor.matmul(
                        pacc[:],
                        oh[:, j, :],
                        mt[:, li, :],
                        start=(j == 0), stop=(j == ntw - 1),
                    )
                seg, wl = w // WPS, w % WPS
                cp = nc.scalar.copy(acc_sb[seg][:, wl, :], pacc[:])
                evicts.append(cp.ins)

            def emit_rs(seg):
                nc.sync.dma_start(
                    acc_dram[seg][:]
                        .rearrange("(w p) c -> p w c", p=128),
                    acc_sb[seg][:])
                nc.gpsimd.collective_compute(
                    "ReduceScatter", ALU.add,
                    ins=[acc_dram[seg][:].opt()],
                    outs=[rs_out[seg][:].opt()],
                    replica_groups=all_group)

            for c in range(cfg.NCHUNK):
                zg = zg_pool.tile([128, cfg.TPC, cfg.RW], bf,
                                  name=f"zg{c}", tag="zg")
                g = nc.gpsimd.dma_gather(
                    zg[:], z_pair[:],
                    idx_sb[:, c * (cfg.CHUNK // 16):(c + 1) * (cfg.CHUNK // 16)],
                    cfg.CHUNK, cfg.CHUNK, cfg.RW,
                    single_packet=cfg.SP,
                    queue_num=c % N_SWDGE_Q,
                )
                add_dep_helper(g.ins, bar.ins, reason="gather after barrier")

                srl_sb = srl_pre.pop(c, None)
                if srl_sb is None:
                    srl_sb = srl_pool.tile([cfg.FEAT, cfg.CHUNK], bf,
                                           name=f"srl{c}", tag="srl")
                    nc.scalar.dma_start(
                        srl_sb[:], srlT[:, c * cfg.CHUNK:(c + 1) * cfg.CHUNK])

                # scores: copy s_src into PSUM, matmul-accumulate s_feat on
                # top (start=False), then lrelu+exp on the Act engine
                psf = psum_sf.tile([128, cfg.TPC, cfg.H], f32,
                                   name=f"psf{c}", tag="psf")
                nc.scalar.copy(psf[:], zg[:, :, cfg.HD:cfg.HDE])
                for t in range(cfg.TPC):
                    nc.tensor.matmul(
                        psf[:, t, :],
                        srl_sb[:, t * 128:(t + 1) * 128],
                        wfe_sb[:],
                        start=False, stop=True,
                    )
                sfs = small.tile([128, cfg.TPC, cfg.H], f32,
                                 name=f"sfs{c}", tag="sfs")
                nc.scalar.copy(sfs[:], psf[:])
                lr = small.tile([128, cfg.TPC, cfg.H], f32,
                                name=f"lr{c}", tag="lr")
                nc.vector.scalar_tensor_tensor(
                    lr[:], sfs[:], 0.01, sfs[:], ALU.mult, ALU.max)

                msg = msg_pool.tile([128, cfg.TPC, cfg.HDE], bf,
                                    name=f"msg{c}", tag="msg")
                nc.scalar.activation(msg[:, :, cfg.HD:], lr[:], AF.Exp)
                nc.vector.tensor_tensor(
                    msg[:, :, 0:cfg.HD].rearrange("p t (d h) -> p t d h", h=cfg.H),
                    zg[:, :, 0:cfg.HD].rearrange("p t (d h) -> p t d h", h=cfg.H),
                    msg[:, :, cfg.HD:].unsqueeze(2)
                        .broadcast_to([128, cfg.TPC, cfg.D, cfg.H]),
                    ALU.mult,
                )
                for t in range(cfg.TPC):
                    msg_tiles[c * cfg.TPC + t] = (msg, t)

                for w in win_by_chunk.get(c, []):
                    emit_window(w)
                    if w % WPS == WPS - 1:
                        rs_pending.append((c + 2, w // WPS))
                while rs_pending and rs_pending[0][0] <= c:
                    emit_rs(rs_pending.pop(0)[1])
            while rs_pending:
                emit_rs(rs_pending.pop(0)[1])

            # ---- finalize: out = msg_tot / max(den, eps) per RS shard ----
            for seg in range(NSEG):
                tot = fin.tile([128, SHT, cfg.HDE], bf,
                               name=f"tot{seg}", tag="tot")
                tl = nc.sync.dma_start(
                    tot[:],
                    rs_out[seg][:].rearrange("(w p) c -> p w c", p=128))
                add_dep_helper(tl.ins, evicts[-1],
                               reason="finalize after last evict")
                den = fin.tile([128, SHT, cfg.H], f32,
                               name=f"den{seg}", tag="den")
                nc.scalar.activation(den[:], tot[:, :, cfg.HD:cfg.HDE],
                                     AF.Copy, bias=1e-9)
                rec = fin.tile([128, SHT, cfg.H], f32,
                               name=f"rec{seg}", tag="rec")
                nc.vector.reciprocal(rec[:], den[:])
                ow = fin.tile([128, SHT, cfg.HD], f32,
                              name=f"ow{seg}", tag="ow")
                nc.vector.tensor_tensor(
                    ow[:].rearrange("p w (d h) -> p w d h", h=cfg.H),
                    tot[:, :, 0:cfg.HD].rearrange("p w (d h) -> p w d h", h=cfg.H),
                    rec[:].unsqueeze(2)
                        .broadcast_to([128, SHT, cfg.D, cfg.H]),
                    ALU.mult,
                )
                nc.sync.dma_start(
                    out_ext[seg * SHR:(seg + 1) * SHR, :]
                        .rearrange("(w p) c -> p w c", p=128),
                    ow[:])

    nc.compile()
    return nc


# --------------------------------------------------------------------------
# host-side preprocessing
# --------------------------------------------------------------------------

def _greedy_windows(cfg: Cfg, dst, pair_of_edge):
    """Assign dst nodes to 80 windows of 125, balancing per-pair edge load.
    Returns slot_of_dst [NP] (window*128 + position)."""
    NW = cfg.NWIN
    # per (dst, pair) degree
    deg = np.zeros((cfg.NP, cfg.NPAIRS), np.int64)
    np.add.at(deg, (dst, pair_of_edge), 1)
    tot = deg.sum(1)
    order = np.argsort(-tot, kind="stable")
    load = np.zeros((NW, cfg.NPAIRS), np.int64)
    count = np.zeros(NW, np.int64)
    win_of = np.empty(cfg.NP, np.int64)
    pos_of = np.empty(cfg.NP, np.int64)
    for d in order:
        cand = load + deg[d][None, :]
        score = cand.max(1) * 1000 + cand.sum(1)
        score[count >= cfg.NODES_PER_WIN] = np.iinfo(np.int64).max
        w = int(np.argmin(score))
        win_of[d] = w
        pos_of[d] = count[w]
        count[w] += 1
        load[w] += deg[d]
    assert (count == cfg.NODES_PER_WIN).all()
    return win_of * 128 + pos_of


def host_prep(cfg: Cfg, h, srl_emb, src, dst, W_fc, W_feat, W_attn):
    H, D = cfg.H, cfg.D

    a = np.asarray(W_attn, F32)[0]
    a_src, a_feat = a[:D], a[2 * D:3 * D]
    W_fc = np.asarray(W_fc, F32)
    Wf_eff = (np.asarray(W_feat, F32).reshape(H, D, cfg.FEAT)
              * a_feat[None, :, None]).sum(1)
    Wz_eff = (W_fc.reshape(H, D, cfg.IN_DIM) * a_src[None, :, None]).sum(1)

    # d-major column order for z: col j <-> (h=j%8, d=j//8)
    perm = np.array([(j % H) * D + j // H for j in range(cfg.HD)], np.int64)
    Wfull = np.concatenate([W_fc.T[:, perm], Wz_eff.T], axis=1)  # [IN_DIM, HDE]
    WfcT_r = np.ascontiguousarray(
        Wfull.reshape(cfg.KC, 128, cfg.HDE).transpose(1, 0, 2)).astype(BF16)
    WfeT_r = np.ascontiguousarray(Wf_eff.T).astype(BF16)
    IotaM = np.tile(np.arange(128, dtype=F32)[None, :], (128, 1)).astype(BF16)

    h_bf = np.zeros((cfg.NA_PAD, cfg.IN_DIM), BF16)
    h_bf[:cfg.NA] = np.asarray(h, F32).astype(BF16)
    srl_bf = np.asarray(srl_emb, F32).astype(BF16)

    src = np.asarray(src, np.int64)
    dst = np.asarray(dst, np.int64)
    pair_of_edge = src // cfg.PAIR_ROWS          # src in padded node space
    slot_of_dst = _greedy_windows(cfg, dst, pair_of_edge)
    win_of_edge = slot_of_dst[dst] // 128

    # core assignment: within (pair, window), alternate between pair cores
    order = np.lexsort((src, win_of_edge, pair_of_edge))
    e_s = order
    pair_s = pair_of_edge[e_s]
    win_s = win_of_edge[e_s]
    key = pair_s * cfg.NWIN + win_s
    # rank within each (pair, window) group
    grp_start = np.r_[True, key[1:] != key[:-1]]
    gidx = np.arange(len(e_s)) - np.maximum.accumulate(
        np.where(grp_start, np.arange(len(e_s)), 0))
    core_s = pair_s * 2 + (gidx % 2)

    in_maps = []
    for c in range(cfg.NCORES):
        sel = core_s == c
        e_c = e_s[sel]                     # sorted by (window, src)
        win_c = win_s[sel]
        cnt = np.bincount(win_c, minlength=cfg.NWIN)
        assert cnt.max() <= cfg.C_WIN, f"C_WIN too small: {cnt.max()}"

        idx = np.zeros(cfg.SLOTS, np.int16)
        dstrel_v = np.full(cfg.SLOTS, -1.0, F32)
        srl_rows = np.zeros((cfg.SLOTS, cfg.FEAT), BF16)
        pos = win_c * cfg.C_WIN + (
            np.arange(len(e_c)) - np.r_[0, np.cumsum(cnt)][win_c])
        row = (src[e_c] - (c // 2) * cfg.PAIR_ROWS).astype(np.int16)
        idx[pos] = row
        dstrel_v[pos] = (slot_of_dst[dst[e_c]] - win_c * 128).astype(F32)
        srl_rows[pos] = srl_bf[e_c]
        # dummy pads: repeat a valid row (idx stays 0 where no edge before;
        # fill window pads with the window's first real row for locality)
        for w in range(cfg.NWIN):
            if cnt[w] < cfg.C_WIN:
                fill = row[np.searchsorted(win_c, w)] if cnt[w] > 0 else 0
                idx[w * cfg.C_WIN + cnt[w]:(w + 1) * cfg.C_WIN] = fill

        srlT_c = np.ascontiguousarray(srl_rows.T)

        # dstrel blocks: per (window, tile-in-window) columns
        dstrel_blk = np.full((128, cfg.NBLK), -1.0, F32)
        slots_v = dstrel_v.reshape(cfg.NTILES, 128).T   # [128, tile]
        for w in range(cfg.NWIN):
            t0, t1 = cfg.win_t0[w], cfg.win_t1[w]
            b0 = int(cfg.win_blk0[w])
            base_shift = np.zeros(t1 - t0, F32)
            # dstrel_v holds slot - win*128 for the edge's own window; for a
            # straddle tile the neighbor window's edges carry their own
            # offset.  Rebase everything to window w:
            for j, t in enumerate(range(t0, t1)):
                col = slots_v[:, t].copy()
                # which window does each slot position belong to?
                slot_ids = t * 128 + np.arange(128)
                w_of_slot = slot_ids // cfg.C_WIN
                valid = col >= 0
                rb = col + (w_of_slot - w) * 128.0
                rb[~valid] = -1.0
                dstrel_blk[:, b0 + j] = rb
        dstrel_c = dstrel_blk.astype(BF16)

        def wrap_idx(arr):  # [SLOTS] -> [128, SLOTS//16]
            wr = arr.reshape(cfg.SLOTS // 16, 16).T
            return np.ascontiguousarray(np.tile(wr, (8, 1)))

        hsl = h_bf[c * cfg.NSH:(c + 1) * cfg.NSH]
        hT_c = np.ascontiguousarray(
            hsl.T.reshape(cfg.KC, 128, cfg.NSH).transpose(1, 0, 2))

        in_maps.append({
            "hT": hT_c,
            "WfcT": WfcT_r,
            "WfeT": WfeT_r,
            "IotaM": IotaM,
            "srlT": srlT_c,
            "dstrel": dstrel_c,
            "idxT": wrap_idx(idx),
        })
    return in_maps, slot_of_dst


def required_c_win(cfg: Cfg, src, dst):
    src = np.asarray(src, np.int64)
    dst = np.asarray(dst, np.int64)
    pair_of_edge = src // cfg.PAIR_ROWS
    slot_of_dst = _greedy_windows(cfg, dst, pair_of_edge)
    win_of_edge = slot_of_dst[dst] // 128
    # worst core count: ceil(pair-window count / 2)
    key = pair_of_edge * cfg.NWIN + win_of_edge
    counts = np.bincount(key, minlength=cfg.NPAIRS * cfg.NWIN)
    need = int(-(-counts.max() // 2))
    return -(-need // 16) * 16


# --------------------------------------------------------------------------
# entry point
# --------------------------------------------------------------------------

_CACHE = {}


def _get_graph(cfg: Cfg):
    key = (cfg.NCORES, cfg.NA_PAD, cfg.NP, cfg.C_WIN, cfg.CHUNK, cfg.SP)
    if key not in _CACHE:
        _CACHE[key] = build_graph(cfg)
    return _CACHE[key]


def kernel(h, srl_emb, src, dst, W_fc, W_feat, W_attn, _trace=False,
           _tmpdir=None):
    src = np.asarray(src)
    dst = np.asarray(dst)
    cfg = Cfg()
    need = required_c_win(cfg, src, dst)
    if need != cfg.C_WIN:
        cfg = Cfg(c_win=need)
    nc = _get_graph(cfg)
    in_maps, slot_of_dst = host_prep(
        cfg, np.asarray(h), np.asarray(srl_emb), src, dst,
        np.asarray(W_fc), np.asarray(W_feat), np.asarray(W_attn))
    res = run_bass_kernel_spmd(nc, in_maps, core_ids=list(range(cfg.NCORES)),
                               trace=_trace, tmpdir=_tmpdir)
    # reassemble: segment s, core c shard covers slots s*2048 + c*256 + [0,256)
    out_slots = np.empty((cfg.SLOT_ROWS, cfg.H, cfg.D), F32)
    NSEG = 2
    SPS = cfg.SLOT_ROWS // NSEG
    SHR = SPS // cfg.NCORES
    for c in range(cfg.NCORES):
        shard = np.asarray(res.results[c]["out"], F32)  # [1280, 256] d-major
        shard = shard.reshape(NSEG, SHR, cfg.D, cfg.H).transpose(0, 1, 3, 2)
        for s in range(NSEG):
            out_slots[s * SPS + c * SHR:s * SPS + (c + 1) * SHR] = shard[s]
    out = out_slots[slot_of_dst]
    if _trace:
        kernel._last_results = res
    return out


# revision 34
# speedup vs baseline: 1.2812x; 1.0312x over previous
"""Trainium2 distributed Bass kernel for the APGAT layer (gnn_message_passing).

v2 strategy (8 NeuronCores, 4 HBM pairs):
  - Nodes are sharded over cores (6272/core); cores (2k, 2k+1) share an HBM
    domain, so each pair holds a PAIR-LOCAL z table of 12544 rows in Shared
    DRAM (written with a rank-dependent dynamic-offset DMA; a tiny pair
    AllReduce acts as the readiness barrier).  12544 < 32768 so gather
    indices fit int16 with a SINGLE table - no A/B split.
  - Edges are assigned to the pair that owns their src node (gathers are
    always pair-local; no z AllGather at all), split evenly between the two
    cores of the pair.  Each core processes ~50k edges spanning ALL dst
    nodes; per-dst partial sums are combined at the end with two pipelined
    8-core ReduceScatters over the 10240 dst slots.
  - dst nodes are relabeled on the host (greedy, degree-balanced) into 80
    windows of 125 nodes so every (core, window) has <= C_WIN edges.
    Edge stream = 80 windows x C_WIN slots, padded with dummy (valid) idx
    rows that dstrel = -1 masks out of the one-hot.
  - Per 896-edge chunk (57 ring descriptors: two calls fit per 128-deep
    SWDGE ring, so desc-gen overlaps the previous call's DMA drain):
    dma_gather of [z | s_src] rows (768 B), s_feat via PE matmul that
    ACCUMULATES onto the s_src scores pre-copied into PSUM (start=False),
    lrelu on DVE (avoids Act activation-table thrashing), exp on Act
    straight into the msg tile, msg = ex (x) z in the DVE 2x mode
    (d-major layout).  Per window: one-hot is_equal + PE segment-sum
    matmuls into PSUM, evicted bf16 to an SBUF accumulator.
  - Two ReduceScatters (NSEG=2) reduce the per-dst partials; RS triggers
    are delayed two chunks so their input DMA is complete before the Pool
    queue reaches them, and the finalize chain is dep-pinned behind the
    last eviction so the scheduler cannot convoy the engine queues on RS
    completion.
  - Softmax max-subtraction is skipped (scores are O(7); validated vs the
    reference, gate is 2e-2).  All-bf16 datapath: fp8 anywhere on the z or
    srl path fails accuracy (the softmax is peaked, so per-element
    quantization error does not average out; dot products keep the
    per-element relative error).
"""

import sys

sys.path.insert(0, "/opt/trn_rl_repo")

import numpy as np
import ml_dtypes

import concourse.bass as bass
import concourse.bacc as bacc
import concourse.mybir as mybir
import concourse.tile as tile
from concourse.bass import ds
from concourse.tile import add_dep_helper
from concourse.bass_utils import run_bass_kernel_spmd

BF16 = ml_dtypes.bfloat16
F8 = ml_dtypes.float8_e4m3fn
F32 = np.float32
AF = mybir.ActivationFunctionType
ALU = mybir.AluOpType

N_SWDGE_Q = 8


class Cfg:
    def __init__(self, ncores=8, na=50000, np_=10000, e=400000, in_dim=512,
                 feat=128, h=8, d=32, c_win=640, chunk=896, single_packet=True):
        self.NCORES = ncores
        self.NPAIRS = ncores // 2
        self.NA = na
        self.NP = np_
        self.E = e
        self.IN_DIM = in_dim
        self.KC = in_dim // 128
        self.FEAT = feat
        self.H = h
        self.D = d
        self.HD = h * d                    # 256
        self.HDE = self.HD + h             # 264: [z | s_src]
        self.RW = 384                      # bf16 row width -> 768 B rows
        nsh = -(-na // ncores)
        self.NSH = -(-nsh // 128) * 128    # nodes per core (6272)
        self.NA_PAD = self.NSH * ncores
        self.NT = self.NSH // 128          # node tiles per core (49)
        self.PAIR_ROWS = 2 * self.NSH      # 12544 (< 32768: int16 idx)
        assert self.PAIR_ROWS <= 32768
        # dst windows: 80 windows x 125 nodes (128 slots each)
        self.NWIN = 80
        self.NODES_PER_WIN = np_ // self.NWIN    # 125
        self.SLOT_ROWS = self.NWIN * 128         # 10240 dst slots
        self.WIN_PER_CORE = self.NWIN // ncores  # 10
        # per (core, window) edge capacity
        assert c_win % 16 == 0
        self.C_WIN = c_win
        self.CHUNK = chunk                 # gather chunk (multiple of 128)
        assert chunk % 128 == 0
        self.SP = single_packet
        slots = self.NWIN * c_win
        self.NCHUNK = -(-slots // chunk)
        self.SLOTS = self.NCHUNK * chunk   # padded stream length
        self.TPC = chunk // 128            # tiles per chunk
        self.NTILES = self.SLOTS // 128
        # per-window tile spans (static given C_WIN)
        self.win_t0 = [(c_win * w) // 128 for w in range(self.NWIN)]
        self.win_t1 = [-(-(c_win * (w + 1)) // 128) for w in range(self.NWIN)]
        self.win_blk0 = np.cumsum([0] + [t1 - t0 for t0, t1 in
                                         zip(self.win_t0, self.win_t1)])
        self.NBLK = int(self.win_blk0[-1])


def build_graph(cfg: Cfg):
    nc = bacc.Bacc("TRN2", target_bir_lowering=False, debug=False,
                   num_devices=cfg.NCORES, num_swdge_queues=N_SWDGE_Q)
    bf = mybir.dt.bfloat16
    f8 = mybir.dt.float8e4
    f32 = mybir.dt.float32
    i16 = mybir.dt.int16
    u8 = mybir.dt.uint8

    # ---- kernel I/O ----
    hT = nc.dram_tensor("hT", [128, cfg.KC, cfg.NSH], bf, kind="ExternalInput")
    WfcT = nc.dram_tensor("WfcT", [128, cfg.KC, cfg.HDE], bf, kind="ExternalInput")
    WfeT = nc.dram_tensor("WfeT", [cfg.FEAT, cfg.H], bf, kind="ExternalInput")
    IotaM = nc.dram_tensor("IotaM", [128, 128], bf, kind="ExternalInput")
    srlT = nc.dram_tensor("srlT", [cfg.FEAT, cfg.SLOTS], bf, kind="ExternalInput")
    dstrel = nc.dram_tensor("dstrel", [128, cfg.NBLK], bf, kind="ExternalInput")
    idxT = nc.dram_tensor("idxT", [128, cfg.SLOTS // 16], i16, kind="ExternalInput")
    out_ext = nc.dram_tensor("out", [2 * cfg.WIN_PER_CORE // 2 * 128, cfg.HD],
                             f32, kind="ExternalOutput")   # [1280, 256]

    pair_groups = [[2 * p, 2 * p + 1] for p in range(cfg.NPAIRS)]
    all_group = [list(range(cfg.NCORES))]
    NSEG = 2                               # pipelined ReduceScatter halves
    WPS = cfg.NWIN // NSEG                 # windows per segment (16)
    SPS = WPS * 128                        # acc rows per segment (2048)
    SHR = SPS // cfg.NCORES                # shard rows per core (256)
    SHT = SHR // 128                       # shard row tiles (2)

    with tile.TileContext(nc) as tc:
        with (
            tc.tile_pool(name="dram", bufs=1, space="DRAM") as dram,
            tc.tile_pool(name="consts", bufs=1) as consts,
            tc.tile_pool(name="psum_sf", bufs=2, space="PSUM") as psum_sf,
            tc.tile_pool(name="psum_acc", bufs=2, space="PSUM") as psum_acc,
            tc.tile_pool(name="srl", bufs=4) as srl_pool,
            tc.tile_pool(name="zg", bufs=7) as zg_pool,
            tc.tile_pool(name="msg", bufs=6) as msg_pool,
            tc.tile_pool(name="oh", bufs=3) as oh_pool,
            tc.tile_pool(name="small", bufs=3) as small,
            tc.tile_pool(name="accsb", bufs=1) as accsb,
            tc.tile_pool(name="fin", bufs=2) as fin,
        ):
            # ---- shared/DRAM scratch ----
            z_pair = dram.tile([cfg.PAIR_ROWS, cfg.RW], bf, addr_space="Shared")
            bar_in = dram.tile([1, 16], bf)
            bar_out = dram.tile([1, 16], bf)
            acc_dram = [dram.tile([SPS, cfg.HDE], bf, name=f"accd{i}")
                        for i in range(NSEG)]
            rs_out = [dram.tile([SHR, cfg.HDE], bf, name=f"rs{i}")
                      for i in range(NSEG)]

            # ---- constants ----
            wfe_sb = consts.tile([cfg.FEAT, cfg.H], bf)
            nc.sync.dma_start(wfe_sb[:], WfeT[:])
            iota_sb = consts.tile([128, 128], bf)
            nc.sync.dma_start(iota_sb[:], IotaM[:])
            idx_sb = consts.tile([128, cfg.SLOTS // 16], i16)
            nc.sync.dma_start(idx_sb[:], idxT[:])
            dst_sb = consts.tile([128, cfg.NBLK], bf)
            nc.sync.dma_start(dst_sb[:], dstrel[:])

            # rank within the pair (0/1) for the z-table write offset
            rank1 = nc.sync.partition_id() % 2
            row_off = rank1 * cfg.NSH

            # ---- phase A: z = [h @ Wfc | s_src] -> pair-shared table ----
            with (
                tc.tile_pool(name="zph_h", bufs=2) as zph_h,
                tc.tile_pool(name="zph_w", bufs=1) as zph_w,
                tc.tile_pool(name="zph_s", bufs=1) as zph_s,
                tc.tile_pool(name="psum_z", bufs=4, space="PSUM") as psum_z,
            ):
                wfc_sb = zph_w.tile([128, cfg.KC, cfg.HDE], bf)
                nc.sync.dma_start(wfc_sb[:], WfcT[:])
                zstage = zph_s.tile([128, cfg.NT, cfg.HDE], bf)
                bounds = [0, 13, 25, 37, cfg.NT]
                for t0, t1 in zip(bounds[:-1], bounds[1:]):
                    nt = t1 - t0
                    hT_sb = zph_h.tile([128, cfg.KC, 13 * 128], bf,
                                       name=f"hT{t0}", tag="hT")
                    nc.sync.dma_start(hT_sb[:, :, 0:nt * 128],
                                      hT[:, :, t0 * 128:t1 * 128])
                    for i in range(nt):
                        pz = psum_z.tile([128, cfg.HDE], f32,
                                         name=f"pz{t0 + i}", tag="pz")
                        for c in range(cfg.KC):
                            nc.tensor.matmul(
                                pz[:],
                                hT_sb[:, c, i * 128:(i + 1) * 128],
                                wfc_sb[:, c, :],
                                start=(c == 0), stop=(c == cfg.KC - 1),
                            )
                        nc.scalar.copy(zstage[:, t0 + i, :], pz[:])
                z_write = nc.sync.dma_start(
                    z_pair[:][ds(row_off, cfg.NSH), 0:cfg.HDE]
                        .rearrange("(t p) r -> p t r", p=128),
                    zstage[:])

            # barrier: pair AllReduce; completes only after both pair cores
            # have finished their z-table writes
            nc.sync.dma_start(bar_in[:], iota_sb[0:1, 0:16])
            bar = nc.gpsimd.collective_compute(
                "AllReduce", ALU.add, ins=[bar_in[:].opt()],
                outs=[bar_out[:].opt()], replica_groups=pair_groups)
            add_dep_helper(bar.ins, z_write.ins, reason="barrier after z write")

            # ---- phase B ----
            # windows grouped by the chunk that completes them
            win_by_chunk = {}
            for w in range(cfg.NWIN):
                lc = (cfg.win_t1[w] - 1) // cfg.TPC
                win_by_chunk.setdefault(lc, []).append(w)

            msg_tiles = {}    # global tile idx -> (msg tile, local idx)
            evicts = []
            rs_pending = []
            acc_sb = [accsb.tile([128, WPS, cfg.HDE], bf, name=f"acc{i}")
                      for i in range(NSEG)]

            # prefetch: first srl chunks + one-hot blocks run during phase A
            srl_pre = {}
            for c in range(3):
                srl_sb = srl_pool.tile([cfg.FEAT, cfg.CHUNK], bf,
                                       name=f"srl{c}", tag="srl")
                nc.scalar.dma_start(
                    srl_sb[:], srlT[:, c * cfg.CHUNK:(c + 1) * cfg.CHUNK])
                srl_pre[c] = srl_sb

            oh_pre = {}

            def build_oh(w):
                t0, t1 = cfg.win_t0[w], cfg.win_t1[w]
                ntw = t1 - t0
                blk0 = int(cfg.win_blk0[w])
                oh = oh_pool.tile([128, ntw, 128], bf, name=f"oh{w}", tag="oh")
                nc.vector.tensor_tensor(
                    oh[:],
                    iota_sb[:].unsqueeze(1).broadcast_to([128, ntw, 128]),
                    dst_sb[:, blk0:blk0 + ntw]
                        .unsqueeze(2).broadcast_to([128, ntw, 128]),
                    ALU.is_equal,
                )
                return oh

            for w in range(3):
                oh_pre[w] = build_oh(w)

            def emit_window(w):
                t0, t1 = cfg.win_t0[w], cfg.win_t1[w]
                ntw = t1 - t0
                oh = oh_pre.pop(w, None)
                if oh is None:
                    oh = build_oh(w)
                pacc = psum_acc.tile([128, cfg.HDE], f32,
                                     name=f"pacc{w}", tag="pacc")
                for j, t in enumerate(range(t0, t1)):
                    mt, li = msg_tiles[t]
                    nc.tensor.matmul(
                        pacc[:],
                        oh[:, j, :],
                        mt[:, li, :],
                        start=(j == 0), stop=(j == ntw - 1),
                    )
                seg, wl = w // WPS, w % WPS
                cp = nc.scalar.copy(acc_sb[seg][:, wl, :], pacc[:])
                evicts.append(cp.ins)

            def emit_rs(seg):
                nc.sync.dma_start(
                    acc_dram[seg][:]
                        .rearrange("(w p) c -> p w c", p=128),
                    acc_sb[seg][:])
                nc.gpsimd.collective_compute(
                    "ReduceScatter", ALU.add,
                    ins=[acc_dram[seg][:].opt()],
                    outs=[rs_out[seg][:].opt()],
                    replica_groups=all_group)

            for c in range(cfg.NCHUNK):
                zg = zg_pool.tile([128, cfg.TPC, cfg.RW], bf,
                                  name=f"zg{c}", tag="zg")
                g = nc.gpsimd.dma_gather(
                    zg[:], z_pair[:],
                    idx_sb[:, c * (cfg.CHUNK // 16):(c + 1) * (cfg.CHUNK // 16)],
                    cfg.CHUNK, cfg.CHUNK, cfg.RW,
                    single_packet=cfg.SP,
                    queue_num=c % N_SWDGE_Q,
                )
                add_dep_helper(g.ins, bar.ins, reason="gather after barrier")

                srl_sb = srl_pre.pop(c, None)
                if srl_sb is None:
                    srl_sb = srl_pool.tile([cfg.FEAT, cfg.CHUNK], bf,
                                           name=f"srl{c}", tag="srl")
                    nc.scalar.dma_start(
                        srl_sb[:], srlT[:, c * cfg.CHUNK:(c + 1) * cfg.CHUNK])

                # scores: copy s_src into PSUM, matmul-accumulate s_feat on
                # top (start=False), then lrelu+exp on the Act engine
                psf = psum_sf.tile([128, cfg.TPC, cfg.H], f32,
                                   name=f"psf{c}", tag="psf")
                nc.scalar.copy(psf[:], zg[:, :, cfg.HD:cfg.HDE])
                for t in range(cfg.TPC):
                    nc.tensor.matmul(
                        psf[:, t, :],
                        srl_sb[:, t * 128:(t + 1) * 128],
                        wfe_sb[:],
                        start=False, stop=True,
                    )
                sfs = small.tile([128, cfg.TPC, cfg.H], f32,
                                 name=f"sfs{c}", tag="sfs")
                nc.scalar.copy(sfs[:], psf[:])
                lr = small.tile([128, cfg.TPC, cfg.H], f32,
                                name=f"lr{c}", tag="lr")
                nc.vector.scalar_tensor_tensor(
                    lr[:], sfs[:], 0.01, sfs[:], ALU.mult, ALU.max)

                msg = msg_pool.tile([128, cfg.TPC, cfg.HDE], bf,
                                    name=f"msg{c}", tag="msg")
                nc.scalar.activation(msg[:, :, cfg.HD:], lr[:], AF.Exp)
                nc.vector.tensor_tensor(
                    msg[:, :, 0:cfg.HD].rearrange("p t (d h) -> p t d h", h=cfg.H),
                    zg[:, :, 0:cfg.HD].rearrange("p t (d h) -> p t d h", h=cfg.H),
                    msg[:, :, cfg.HD:].unsqueeze(2)
                        .broadcast_to([128, cfg.TPC, cfg.D, cfg.H]),
                    ALU.mult,
                )
                for t in range(cfg.TPC):
                    msg_tiles[c * cfg.TPC + t] = (msg, t)

                for w in win_by_chunk.get(c, []):
                    emit_window(w)
                    if w % WPS == WPS - 1:
                        rs_pending.append((c + 2, w // WPS))
                while rs_pending and rs_pending[0][0] <= c:
                    emit_rs(rs_pending.pop(0)[1])
            while rs_pending:
                emit_rs(rs_pending.pop(0)[1])

            # ---- finalize: out = msg_tot / max(den, eps) per RS shard ----
            for seg in range(NSEG):
                tot = fin.tile([128, SHT, cfg.HDE], bf,
                               name=f"tot{seg}", tag="tot")
                tl = nc.sync.dma_start(
                    tot[:],
                    rs_out[seg][:].rearrange("(w p) c -> p w c", p=128))
                add_dep_helper(tl.ins, evicts[-1],
                               reason="finalize after last evict")
                den = fin.tile([128, SHT, cfg.H], f32,
                               name=f"den{seg}", tag="den")
                nc.scalar.activation(den[:], tot[:, :, cfg.HD:cfg.HDE],
                                     AF.Copy, bias=1e-9)
                rec = fin.tile([128, SHT, cfg.H], f32,
                               name=f"rec{seg}", tag="rec")
                nc.vector.reciprocal(rec[:], den[:])
                ow = fin.tile([128, SHT, cfg.HD], f32,
                              name=f"ow{seg}", tag="ow")
                nc.vector.tensor_tensor(
                    ow[:].rearrange("p w (d h) -> p w d h", h=cfg.H),
                    tot[:, :, 0:cfg.HD].rearrange("p w (d h) -> p w d h", h=cfg.H),
                    rec[:].unsqueeze(2)
                        .broadcast_to([128, SHT, cfg.D, cfg.H]),
                    ALU.mult,
                )
                nc.sync.dma_start(
                    out_ext[seg * SHR:(seg + 1) * SHR, :]
                        .rearrange("(w p) c -> p w c", p=128),
                    ow[:])

    nc.compile()
    return nc


# --------------------------------------------------------------------------
# host-side preprocessing
# --------------------------------------------------------------------------

def _greedy_windows(cfg: Cfg, dst, pair_of_edge):
    """Assign dst nodes to 80 windows of 125, balancing per-pair edge load.
    Returns slot_of_dst [NP] (window*128 + position)."""
    NW = cfg.NWIN
    # per (dst, pair) degree
    deg = np.zeros((cfg.NP, cfg.NPAIRS), np.int64)
    np.add.at(deg, (dst, pair_of_edge), 1)
    tot = deg.sum(1)
    order = np.argsort(-tot, kind="stable")
    load = np.zeros((NW, cfg.NPAIRS), np.int64)
    count = np.zeros(NW, np.int64)
    win_of = np.empty(cfg.NP, np.int64)
    pos_of = np.empty(cfg.NP, np.int64)
    for d in order:
        cand = load + deg[d][None, :]
        score = cand.max(1) * 1000 + cand.sum(1)
        score[count >= cfg.NODES_PER_WIN] = np.iinfo(np.int64).max
        w = int(np.argmin(score))
        win_of[d] = w
        pos_of[d] = count[w]
        count[w] += 1
        load[w] += deg[d]
    assert (count == cfg.NODES_PER_WIN).all()
    return win_of * 128 + pos_of


def host_prep(cfg: Cfg, h, srl_emb, src, dst, W_fc, W_feat, W_attn):
    H, D = cfg.H, cfg.D

    a = np.asarray(W_attn, F32)[0]
    a_src, a_feat = a[:D], a[2 * D:3 * D]
    W_fc = np.asarray(W_fc, F32)
    Wf_eff = (np.asarray(W_feat, F32).reshape(H, D, cfg.FEAT)
              * a_feat[None, :, None]).sum(1)
    Wz_eff = (W_fc.reshape(H, D, cfg.IN_DIM) * a_src[None, :, None]).sum(1)

    # d-major column order for z: col j <-> (h=j%8, d=j//8)
    perm = np.array([(j % H) * D + j // H for j in range(cfg.HD)], np.int64)
    Wfull = np.concatenate([W_fc.T[:, perm], Wz_eff.T], axis=1)  # [IN_DIM, HDE]
    WfcT_r = np.ascontiguousarray(
        Wfull.reshape(cfg.KC, 128, cfg.HDE).transpose(1, 0, 2)).astype(BF16)
    WfeT_r = np.ascontiguousarray(Wf_eff.T).astype(BF16)
    IotaM = np.tile(np.arange(128, dtype=F32)[None, :], (128, 1)).astype(BF16)

    h_bf = np.zeros((cfg.NA_PAD, cfg.IN_DIM), BF16)
    h_bf[:cfg.NA] = np.asarray(h, F32).astype(BF16)
    srl_bf = np.asarray(srl_emb, F32).astype(BF16)

    src = np.asarray(src, np.int64)
    dst = np.asarray(dst, np.int64)
    pair_of_edge = src // cfg.PAIR_ROWS          # src in padded node space
    slot_of_dst = _greedy_windows(cfg, dst, pair_of_edge)
    win_of_edge = slot_of_dst[dst] // 128

    # core assignment: within (pair, window), alternate between pair cores
    order = np.lexsort((src, win_of_edge, pair_of_edge))
    e_s = order
    pair_s = pair_of_edge[e_s]
    win_s = win_of_edge[e_s]
    key = pair_s * cfg.NWIN + win_s
    # rank within each (pair, window) group
    grp_start = np.r_[True, key[1:] != key[:-1]]
    gidx = np.arange(len(e_s)) - np.maximum.accumulate(
        np.where(grp_start, np.arange(len(e_s)), 0))
    core_s = pair_s * 2 + (gidx % 2)

    in_maps = []
    for c in range(cfg.NCORES):
        sel = core_s == c
        e_c = e_s[sel]                     # sorted by (window, src)
        win_c = win_s[sel]
        cnt = np.bincount(win_c, minlength=cfg.NWIN)
        assert cnt.max() <= cfg.C_WIN, f"C_WIN too small: {cnt.max()}"

        idx = np.zeros(cfg.SLOTS, np.int16)
        dstrel_v = np.full(cfg.SLOTS, -1.0, F32)
        srl_rows = np.zeros((cfg.SLOTS, cfg.FEAT), BF16)
        pos = win_c * cfg.C_WIN + (
            np.arange(len(e_c)) - np.r_[0, np.cumsum(cnt)][win_c])
        row = (src[e_c] - (c // 2) * cfg.PAIR_ROWS).astype(np.int16)
        idx[pos] = row
        dstrel_v[pos] = (slot_of_dst[dst[e_c]] - win_c * 128).astype(F32)
        srl_rows[pos] = srl_bf[e_c]
        # dummy pads: repeat a valid row (idx stays 0 where no edge before;
        # fill window pads with the window's first real row for locality)
        for w in range(cfg.NWIN):
            if cnt[w] < cfg.C_WIN:
                fill = row[np.searchsorted(win_c, w)] if cnt[w] > 0 else 0
                idx[w * cfg.C_WIN + cnt[w]:(w + 1) * cfg.C_WIN] = fill

        srlT_c = np.ascontiguousarray(srl_rows.T)

        # dstrel blocks: per (window, tile-in-window) columns
        dstrel_blk = np.full((128, cfg.NBLK), -1.0, F32)
        slots_v = dstrel_v.reshape(cfg.NTILES, 128).T   # [128, tile]
        for w in range(cfg.NWIN):
            t0, t1 = cfg.win_t0[w], cfg.win_t1[w]
            b0 = int(cfg.win_blk0[w])
            base_shift = np.zeros(t1 - t0, F32)
            # dstrel_v holds slot - win*128 for the edge's own window; for a
            # straddle tile the neighbor window's edges carry their own
            # offset.  Rebase everything to window w:
            for j, t in enumerate(range(t0, t1)):
                col = slots_v[:, t].copy()
                # which window does each slot position belong to?
                slot_ids = t * 128 + np.arange(128)
                w_of_slot = slot_ids // cfg.C_WIN
                valid = col >= 0
                rb = col + (w_of_slot - w) * 128.0
                rb[~valid] = -1.0
                dstrel_blk[:, b0 + j] = rb
        dstrel_c = dstrel_blk.astype(BF16)

        def wrap_idx(arr):  # [SLOTS] -> [128, SLOTS//16]
            wr = arr.reshape(cfg.SLOTS // 16, 16).T
            return np.ascontiguousarray(np.tile(wr, (8, 1)))

        hsl = h_bf[c * cfg.NSH:(c + 1) * cfg.NSH]
        hT_c = np.ascontiguousarray(
            hsl.T.reshape(cfg.KC, 128, cfg.NSH).transpose(1, 0, 2))

        in_maps.append({
            "hT": hT_c,
            "WfcT": WfcT_r,
            "WfeT": WfeT_r,
            "IotaM": IotaM,
            "srlT": srlT_c,
            "dstrel": dstrel_c,
            "idxT": wrap_idx(idx),
        })
    return in_maps, slot_of_dst


def required_c_win(cfg: Cfg, src, dst):
    src = np.asarray(src, np.int64)
    dst = np.asarray(dst, np.int64)
    pair_of_edge = src // cfg.PAIR_ROWS
    slot_of_dst = _greedy_windows(cfg, dst, pair_of_edge)
    win_of_edge = slot_of_dst[dst] // 128
    # worst core count: ceil(pair-window count / 2)
    key = pair_of_edge * cfg.NWIN + win_of_edge
    counts = np.bincount(key, minlength=cfg.NPAIRS * cfg.NWIN)
    need = int(-(-counts.max() // 2))
    return -(-need // 16) * 16


# --------------------------------------------------------------------------
# entry point
# --------------------------------------------------------------------------

_CACHE = {}


def _get_graph(cfg: Cfg):
    key = (cfg.NCORES, cfg.NA_PAD, cfg.NP, cfg.C_WIN, cfg.CHUNK, cfg.SP)
    if key not in _CACHE:
        _CACHE[key] = build_graph(cfg)
    return _CACHE[key]


def kernel(h, srl_emb, src, dst, W_fc, W_feat, W_attn, _trace=False,
           _tmpdir=None):
    src = np.asarray(src)
    dst = np.asarray(dst)
    cfg = Cfg()
    need = required_c_win(cfg, src, dst)
    if need != cfg.C_WIN:
        cfg = Cfg(c_win=need)
    nc = _get_graph(cfg)
    in_maps, slot_of_dst = host_prep(
        cfg, np.asarray(h), np.asarray(srl_emb), src, dst,
        np.asarray(W_fc), np.asarray(W_feat), np.asarray(W_attn))
    res = run_bass_kernel_spmd(nc, in_maps, core_ids=list(range(cfg.NCORES)),
                               trace=_trace, tmpdir=_tmpdir)
    # reassemble: segment s, core c shard covers slots s*2048 + c*256 + [0,256)
    out_slots = np.empty((cfg.SLOT_ROWS, cfg.H, cfg.D), F32)
    NSEG = 2
    SPS = cfg.SLOT_ROWS // NSEG
    SHR = SPS // cfg.NCORES
    for c in range(cfg.NCORES):
        shard = np.asarray(res.results[c]["out"], F32)  # [1280, 256] d-major
        shard = shard.reshape(NSEG, SHR, cfg.D, cfg.H).transpose(0, 1, 3, 2)
        for s in range(NSEG):
            out_slots[s * SPS + c * SHR:s * SPS + (c + 1) * SHR] = shard[s]
    out = out_slots[slot_of_dst]
    if _trace:
        kernel._last_results = res
    return out


# revision 37
# speedup vs baseline: 1.2921x; 1.0085x over previous
"""Trainium2 distributed Bass kernel for the APGAT layer (gnn_message_passing).

v2 strategy (8 NeuronCores, 4 HBM pairs):
  - Nodes are sharded over cores (6272/core); cores (2k, 2k+1) share an HBM
    domain, so each pair holds a PAIR-LOCAL z table of 12544 rows in Shared
    DRAM (written with a rank-dependent dynamic-offset DMA; a tiny pair
    AllReduce acts as the readiness barrier).  12544 < 32768 so gather
    indices fit int16 with a SINGLE table - no A/B split.
  - Edges are assigned to the pair that owns their src node (gathers are
    always pair-local; no z AllGather at all), split evenly between the two
    cores of the pair.  Each core processes ~50k edges spanning ALL dst
    nodes; per-dst partial sums are combined at the end with two pipelined
    8-core ReduceScatters over the 10240 dst slots.
  - dst nodes are relabeled on the host (greedy, degree-balanced) into 80
    windows of 125 nodes so every (core, window) has <= C_WIN edges.
    Edge stream = 80 windows x C_WIN slots, padded with dummy (valid) idx
    rows that dstrel = -1 masks out of the one-hot.
  - Per 1792-edge chunk: SWDGE dma_gather of [z | s_src] rows (768 B),
    s_feat via PE (srl stationary), scores -> exp, msg = ex (x) z in the
    DVE 2x mode (d-major layout).  Per window: one-hot is_equal + PE
    segment-sum matmuls into PSUM, evicted bf16 to an SBUF accumulator.
  - Softmax max-subtraction is skipped (scores are O(7); validated vs the
    reference, gate is 2e-2).
"""

import sys

sys.path.insert(0, "/opt/trn_rl_repo")

import numpy as np
import ml_dtypes

import concourse.bass as bass
import concourse.bacc as bacc
import concourse.mybir as mybir
import concourse.tile as tile
from concourse.bass import ds
from concourse.tile import add_dep_helper
from concourse.bass_utils import run_bass_kernel_spmd

BF16 = ml_dtypes.bfloat16
F8 = ml_dtypes.float8_e4m3fn
F32 = np.float32
AF = mybir.ActivationFunctionType
ALU = mybir.AluOpType

N_SWDGE_Q = 8


class Cfg:
    def __init__(self, ncores=8, na=50000, np_=10000, e=400000, in_dim=512,
                 feat=128, h=8, d=32, c_win=640, chunk=896, single_packet=True):
        self.NCORES = ncores
        self.NPAIRS = ncores // 2
        self.NA = na
        self.NP = np_
        self.E = e
        self.IN_DIM = in_dim
        self.KC = in_dim // 128
        self.FEAT = feat
        self.H = h
        self.D = d
        self.HD = h * d                    # 256
        self.HDE = self.HD + h             # 264: [z | s_src]
        self.RW = 384                      # bf16 row width -> 768 B rows
        nsh = -(-na // ncores)
        self.NSH = -(-nsh // 128) * 128    # nodes per core (6272)
        self.NA_PAD = self.NSH * ncores
        self.NT = self.NSH // 128          # node tiles per core (49)
        self.PAIR_ROWS = 2 * self.NSH      # 12544 (< 32768: int16 idx)
        assert self.PAIR_ROWS <= 32768
        # dst windows: 80 windows x 125 nodes (128 slots each)
        self.NWIN = 80
        self.NODES_PER_WIN = np_ // self.NWIN    # 125
        self.SLOT_ROWS = self.NWIN * 128         # 10240 dst slots
        self.WIN_PER_CORE = self.NWIN // ncores  # 10
        # per (core, window) edge capacity
        assert c_win % 16 == 0
        self.C_WIN = c_win
        self.CHUNK = chunk                 # gather chunk (multiple of 128)
        assert chunk % 128 == 0
        self.SP = single_packet
        slots = self.NWIN * c_win
        self.NCHUNK = -(-slots // chunk)
        self.SLOTS = self.NCHUNK * chunk   # padded stream length
        self.TPC = chunk // 128            # tiles per chunk
        self.NTILES = self.SLOTS // 128
        # per-window tile spans (static given C_WIN)
        self.win_t0 = [(c_win * w) // 128 for w in range(self.NWIN)]
        self.win_t1 = [-(-(c_win * (w + 1)) // 128) for w in range(self.NWIN)]
        self.win_blk0 = np.cumsum([0] + [t1 - t0 for t0, t1 in
                                         zip(self.win_t0, self.win_t1)])
        self.NBLK = int(self.win_blk0[-1])


def build_graph(cfg: Cfg):
    nc = bacc.Bacc("TRN2", target_bir_lowering=False, debug=False,
                   num_devices=cfg.NCORES, num_swdge_queues=N_SWDGE_Q)
    bf = mybir.dt.bfloat16
    f8 = mybir.dt.float8e4
    f32 = mybir.dt.float32
    i16 = mybir.dt.int16
    u8 = mybir.dt.uint8

    # ---- kernel I/O ----
    hT = nc.dram_tensor("hT", [128, cfg.KC, cfg.NSH], bf, kind="ExternalInput")
    WfcT = nc.dram_tensor("WfcT", [128, cfg.KC, cfg.HDE], bf, kind="ExternalInput")
    WfeT = nc.dram_tensor("WfeT", [cfg.FEAT, cfg.H], bf, kind="ExternalInput")
    IotaM = nc.dram_tensor("IotaM", [128, 128], bf, kind="ExternalInput")
    srlT = nc.dram_tensor("srlT", [cfg.FEAT, cfg.SLOTS], bf, kind="ExternalInput")
    dstrel = nc.dram_tensor("dstrel", [128, cfg.NBLK], bf, kind="ExternalInput")
    idxT = nc.dram_tensor("idxT", [128, cfg.SLOTS // 16], i16, kind="ExternalInput")
    out_ext = nc.dram_tensor("out", [2 * cfg.WIN_PER_CORE // 2 * 128, cfg.HD],
                             f32, kind="ExternalOutput")   # [1280, 256]

    pair_groups = [[2 * p, 2 * p + 1] for p in range(cfg.NPAIRS)]
    all_group = [list(range(cfg.NCORES))]
    NSEG = 2                               # pipelined ReduceScatter halves
    WPS = cfg.NWIN // NSEG                 # windows per segment (16)
    SPS = WPS * 128                        # acc rows per segment (2048)
    SHR = SPS // cfg.NCORES                # shard rows per core (256)
    SHT = SHR // 128                       # shard row tiles (2)

    with tile.TileContext(nc) as tc:
        with (
            tc.tile_pool(name="dram", bufs=1, space="DRAM") as dram,
            tc.tile_pool(name="consts", bufs=1) as consts,
            tc.tile_pool(name="psum_sf", bufs=2, space="PSUM") as psum_sf,
            tc.tile_pool(name="psum_acc", bufs=2, space="PSUM") as psum_acc,
            tc.tile_pool(name="srl", bufs=4) as srl_pool,
            tc.tile_pool(name="zg", bufs=7) as zg_pool,
            tc.tile_pool(name="msg", bufs=6) as msg_pool,
            tc.tile_pool(name="oh", bufs=3) as oh_pool,
            tc.tile_pool(name="small", bufs=3) as small,
            tc.tile_pool(name="accsb", bufs=1) as accsb,
            tc.tile_pool(name="fin", bufs=2) as fin,
        ):
            # ---- shared/DRAM scratch ----
            z_pair = dram.tile([cfg.PAIR_ROWS, cfg.RW], bf, addr_space="Shared")
            bar_in = dram.tile([1, 16], bf)
            bar_out = dram.tile([1, 16], bf)
            acc_dram = [dram.tile([SPS, cfg.HDE], bf, name=f"accd{i}")
                        for i in range(NSEG)]
            rs_out = [dram.tile([SHR, cfg.HDE], bf, name=f"rs{i}")
                      for i in range(NSEG)]

            # ---- constants ----
            wfe_sb = consts.tile([cfg.FEAT, cfg.H], bf)
            nc.sync.dma_start(wfe_sb[:], WfeT[:])
            iota_sb = consts.tile([128, 128], bf)
            nc.sync.dma_start(iota_sb[:], IotaM[:])
            idx_sb = consts.tile([128, cfg.SLOTS // 16], i16)
            nc.sync.dma_start(idx_sb[:], idxT[:])
            dst_sb = consts.tile([128, cfg.NBLK], bf)
            nc.sync.dma_start(dst_sb[:], dstrel[:])

            # rank within the pair (0/1) for the z-table write offset
            rank1 = nc.sync.partition_id() % 2
            row_off = rank1 * cfg.NSH

            # ---- phase A: z = [h @ Wfc | s_src] -> pair-shared table ----
            with (
                tc.tile_pool(name="zph_h", bufs=2) as zph_h,
                tc.tile_pool(name="zph_w", bufs=1) as zph_w,
                tc.tile_pool(name="zph_s", bufs=1) as zph_s,
                tc.tile_pool(name="psum_z", bufs=4, space="PSUM") as psum_z,
            ):
                wfc_sb = zph_w.tile([128, cfg.KC, cfg.HDE], bf)
                nc.sync.dma_start(wfc_sb[:], WfcT[:])
                zstage = zph_s.tile([128, cfg.NT, cfg.HDE], bf)
                bounds = [0, 13, 25, 37, cfg.NT]
                for t0, t1 in zip(bounds[:-1], bounds[1:]):
                    nt = t1 - t0
                    hT_sb = zph_h.tile([128, cfg.KC, 13 * 128], bf,
                                       name=f"hT{t0}", tag="hT")
                    nc.sync.dma_start(hT_sb[:, :, 0:nt * 128],
                                      hT[:, :, t0 * 128:t1 * 128])
                    for i in range(nt):
                        pz = psum_z.tile([128, cfg.HDE], f32,
                                         name=f"pz{t0 + i}", tag="pz")
                        for c in range(cfg.KC):
                            nc.tensor.matmul(
                                pz[:],
                                hT_sb[:, c, i * 128:(i + 1) * 128],
                                wfc_sb[:, c, :],
                                start=(c == 0), stop=(c == cfg.KC - 1),
                            )
                        nc.scalar.copy(zstage[:, t0 + i, :], pz[:])
                z_write = nc.sync.dma_start(
                    z_pair[:][ds(row_off, cfg.NSH), 0:cfg.HDE]
                        .rearrange("(t p) r -> p t r", p=128),
                    zstage[:])

            # barrier: pair AllReduce; completes only after both pair cores
            # have finished their z-table writes
            nc.sync.dma_start(bar_in[:], iota_sb[0:1, 0:16])
            bar = nc.gpsimd.collective_compute(
                "AllReduce", ALU.add, ins=[bar_in[:].opt()],
                outs=[bar_out[:].opt()], replica_groups=pair_groups)
            add_dep_helper(bar.ins, z_write.ins, reason="barrier after z write")

            # ---- phase B ----
            # windows grouped by the chunk that completes them
            win_by_chunk = {}
            for w in range(cfg.NWIN):
                lc = (cfg.win_t1[w] - 1) // cfg.TPC
                win_by_chunk.setdefault(lc, []).append(w)

            msg_tiles = {}    # global tile idx -> (msg tile, local idx)
            evicts = []
            rs_pending = []
            acc_sb = [accsb.tile([128, WPS, cfg.HDE], bf, name=f"acc{i}")
                      for i in range(NSEG)]

            # prefetch: first srl chunks + one-hot blocks run during phase A
            srl_pre = {}
            for c in range(3):
                srl_sb = srl_pool.tile([cfg.FEAT, cfg.CHUNK], bf,
                                       name=f"srl{c}", tag="srl")
                nc.scalar.dma_start(
                    srl_sb[:], srlT[:, c * cfg.CHUNK:(c + 1) * cfg.CHUNK])
                srl_pre[c] = srl_sb

            oh_pre = {}

            def build_oh(w):
                t0, t1 = cfg.win_t0[w], cfg.win_t1[w]
                ntw = t1 - t0
                blk0 = int(cfg.win_blk0[w])
                oh = oh_pool.tile([128, ntw, 128], bf, name=f"oh{w}", tag="oh")
                nc.vector.tensor_tensor(
                    oh[:],
                    iota_sb[:].unsqueeze(1).broadcast_to([128, ntw, 128]),
                    dst_sb[:, blk0:blk0 + ntw]
                        .unsqueeze(2).broadcast_to([128, ntw, 128]),
                    ALU.is_equal,
                )
                return oh

            for w in range(3):
                oh_pre[w] = build_oh(w)

            def emit_window(w):
                t0, t1 = cfg.win_t0[w], cfg.win_t1[w]
                ntw = t1 - t0
                oh = oh_pre.pop(w, None)
                if oh is None:
                    oh = build_oh(w)
                pacc = psum_acc.tile([128, cfg.HDE], f32,
                                     name=f"pacc{w}", tag="pacc")
                for j, t in enumerate(range(t0, t1)):
                    mt, li = msg_tiles[t]
                    nc.tensor.matmul(
                        pacc[:],
                        oh[:, j, :],
                        mt[:, li, :],
                        start=(j == 0), stop=(j == ntw - 1),
                    )
                seg, wl = w // WPS, w % WPS
                cp = nc.scalar.copy(acc_sb[seg][:, wl, :], pacc[:])
                evicts.append(cp.ins)

            def emit_rs(seg):
                nc.sync.dma_start(
                    acc_dram[seg][:]
                        .rearrange("(w p) c -> p w c", p=128),
                    acc_sb[seg][:])
                nc.gpsimd.collective_compute(
                    "ReduceScatter", ALU.add,
                    ins=[acc_dram[seg][:].opt()],
                    outs=[rs_out[seg][:].opt()],
                    replica_groups=all_group)

            for c in range(cfg.NCHUNK):
                zg = zg_pool.tile([128, cfg.TPC, cfg.RW], bf,
                                  name=f"zg{c}", tag="zg")
                g = nc.gpsimd.dma_gather(
                    zg[:], z_pair[:],
                    idx_sb[:, c * (cfg.CHUNK // 16):(c + 1) * (cfg.CHUNK // 16)],
                    cfg.CHUNK, cfg.CHUNK, cfg.RW,
                    single_packet=cfg.SP,
                    queue_num=c % N_SWDGE_Q,
                )
                add_dep_helper(g.ins, bar.ins, reason="gather after barrier")

                srl_sb = srl_pre.pop(c, None)
                if srl_sb is None:
                    srl_sb = srl_pool.tile([cfg.FEAT, cfg.CHUNK], bf,
                                           name=f"srl{c}", tag="srl")
                    nc.scalar.dma_start(
                        srl_sb[:], srlT[:, c * cfg.CHUNK:(c + 1) * cfg.CHUNK])

                # scores: copy s_src into PSUM, matmul-accumulate s_feat on
                # top (start=False), then lrelu+exp on the Act engine
                psf = psum_sf.tile([128, cfg.TPC, cfg.H], f32,
                                   name=f"psf{c}", tag="psf")
                nc.scalar.copy(psf[:], zg[:, :, cfg.HD:cfg.HDE])
                for t in range(cfg.TPC):
                    nc.tensor.matmul(
                        psf[:, t, :],
                        srl_sb[:, t * 128:(t + 1) * 128],
                        wfe_sb[:],
                        start=False, stop=True,
                    )
                sfs = small.tile([128, cfg.TPC, cfg.H], f32,
                                 name=f"sfs{c}", tag="sfs")
                nc.scalar.copy(sfs[:], psf[:])
                lr = small.tile([128, cfg.TPC, cfg.H], f32,
                                name=f"lr{c}", tag="lr")
                nc.vector.scalar_tensor_tensor(
                    lr[:], sfs[:], 0.01, sfs[:], ALU.mult, ALU.max)

                msg = msg_pool.tile([128, cfg.TPC, cfg.HDE], bf,
                                    name=f"msg{c}", tag="msg")
                nc.scalar.activation(msg[:, :, cfg.HD:], lr[:], AF.Exp)
                nc.vector.tensor_tensor(
                    msg[:, :, 0:cfg.HD].rearrange("p t (d h) -> p t d h", h=cfg.H),
                    zg[:, :, 0:cfg.HD].rearrange("p t (d h) -> p t d h", h=cfg.H),
                    msg[:, :, cfg.HD:].unsqueeze(2)
                        .broadcast_to([128, cfg.TPC, cfg.D, cfg.H]),
                    ALU.mult,
                )
                for t in range(cfg.TPC):
                    msg_tiles[c * cfg.TPC + t] = (msg, t)

                for w in win_by_chunk.get(c, []):
                    emit_window(w)
                    if w % WPS == WPS - 1:
                        rs_pending.append((c + 2, w // WPS))
                while rs_pending and rs_pending[0][0] <= c:
                    emit_rs(rs_pending.pop(0)[1])
            while rs_pending:
                emit_rs(rs_pending.pop(0)[1])

            # ---- finalize: out = msg_tot / max(den, eps) per RS shard ----
            for seg in range(NSEG):
                tot = fin.tile([128, SHT, cfg.HDE], bf,
                               name=f"tot{seg}", tag="tot")
                tl = nc.sync.dma_start(
                    tot[:],
                    rs_out[seg][:].rearrange("(w p) c -> p w c", p=128))
                add_dep_helper(tl.ins, evicts[-1],
                               reason="finalize after last evict")
                den = fin.tile([128, SHT, cfg.H], f32,
                               name=f"den{seg}", tag="den")
                nc.scalar.activation(den[:], tot[:, :, cfg.HD:cfg.HDE],
                                     AF.Copy, bias=1e-9)
                rec = fin.tile([128, SHT, cfg.H], f32,
                               name=f"rec{seg}", tag="rec")
                nc.vector.reciprocal(rec[:], den[:])
                ow = fin.tile([128, SHT, cfg.HD], f32,
                              name=f"ow{seg}", tag="ow")
                nc.vector.tensor_tensor(
                    ow[:].rearrange("p w (d h) -> p w d h", h=cfg.H),
                    tot[:, :, 0:cfg.HD].rearrange("p w (d h) -> p w d h", h=cfg.H),
                    rec[:].unsqueeze(2)
                        .broadcast_to([128, SHT, cfg.D, cfg.H]),
                    ALU.mult,
                )
                nc.sync.dma_start(
                    out_ext[seg * SHR:(seg + 1) * SHR, :]
                        .rearrange("(w p) c -> p w c", p=128),
                    ow[:])

    nc.compile()
    return nc


# --------------------------------------------------------------------------
# host-side preprocessing
# --------------------------------------------------------------------------

def _greedy_windows(cfg: Cfg, dst, pair_of_edge):
    """Assign dst nodes to 80 windows of 125, balancing per-pair edge load.
    Returns slot_of_dst [NP] (window*128 + position)."""
    NW = cfg.NWIN
    # per (dst, pair) degree
    deg = np.zeros((cfg.NP, cfg.NPAIRS), np.int64)
    np.add.at(deg, (dst, pair_of_edge), 1)
    tot = deg.sum(1)
    order = np.argsort(-tot, kind="stable")
    load = np.zeros((NW, cfg.NPAIRS), np.int64)
    count = np.zeros(NW, np.int64)
    win_of = np.empty(cfg.NP, np.int64)
    pos_of = np.empty(cfg.NP, np.int64)
    for d in order:
        cand = load + deg[d][None, :]
        score = cand.max(1) * 1000 + cand.sum(1)
        score[count >= cfg.NODES_PER_WIN] = np.iinfo(np.int64).max
        w = int(np.argmin(score))
        win_of[d] = w
        pos_of[d] = count[w]
        count[w] += 1
        load[w] += deg[d]
    assert (count == cfg.NODES_PER_WIN).all()
    return win_of * 128 + pos_of


def host_prep(cfg: Cfg, h, srl_emb, src, dst, W_fc, W_feat, W_attn):
    H, D = cfg.H, cfg.D

    a = np.asarray(W_attn, F32)[0]
    a_src, a_feat = a[:D], a[2 * D:3 * D]
    W_fc = np.asarray(W_fc, F32)
    Wf_eff = (np.asarray(W_feat, F32).reshape(H, D, cfg.FEAT)
              * a_feat[None, :, None]).sum(1)
    Wz_eff = (W_fc.reshape(H, D, cfg.IN_DIM) * a_src[None, :, None]).sum(1)

    # d-major column order for z: col j <-> (h=j%8, d=j//8)
    perm = np.array([(j % H) * D + j // H for j in range(cfg.HD)], np.int64)
    Wfull = np.concatenate([W_fc.T[:, perm], Wz_eff.T], axis=1)  # [IN_DIM, HDE]
    WfcT_r = np.ascontiguousarray(
        Wfull.reshape(cfg.KC, 128, cfg.HDE).transpose(1, 0, 2)).astype(BF16)
    WfeT_r = np.ascontiguousarray(Wf_eff.T).astype(BF16)
    IotaM = np.tile(np.arange(128, dtype=F32)[None, :], (128, 1)).astype(BF16)

    h_bf = np.zeros((cfg.NA_PAD, cfg.IN_DIM), BF16)
    h_bf[:cfg.NA] = np.asarray(h, F32).astype(BF16)
    srl_bf = np.asarray(srl_emb, F32).astype(BF16)

    src = np.asarray(src, np.int64)
    dst = np.asarray(dst, np.int64)
    pair_of_edge = src // cfg.PAIR_ROWS          # src in padded node space
    slot_of_dst = _greedy_windows(cfg, dst, pair_of_edge)
    win_of_edge = slot_of_dst[dst] // 128

    # core assignment: within (pair, window), alternate between pair cores
    order = np.lexsort((src, win_of_edge, pair_of_edge))
    e_s = order
    pair_s = pair_of_edge[e_s]
    win_s = win_of_edge[e_s]
    key = pair_s * cfg.NWIN + win_s
    # rank within each (pair, window) group
    grp_start = np.r_[True, key[1:] != key[:-1]]
    gidx = np.arange(len(e_s)) - np.maximum.accumulate(
        np.where(grp_start, np.arange(len(e_s)), 0))
    core_s = pair_s * 2 + (gidx % 2)

    in_maps = []
    for c in range(cfg.NCORES):
        sel = core_s == c
        e_c = e_s[sel]                     # sorted by (window, src)
        win_c = win_s[sel]
        cnt = np.bincount(win_c, minlength=cfg.NWIN)
        assert cnt.max() <= cfg.C_WIN, f"C_WIN too small: {cnt.max()}"

        idx = np.zeros(cfg.SLOTS, np.int16)
        dstrel_v = np.full(cfg.SLOTS, -1.0, F32)
        srl_rows = np.zeros((cfg.SLOTS, cfg.FEAT), BF16)
        pos = win_c * cfg.C_WIN + (
            np.arange(len(e_c)) - np.r_[0, np.cumsum(cnt)][win_c])
        row = (src[e_c] - (c // 2) * cfg.PAIR_ROWS).astype(np.int16)
        idx[pos] = row
        dstrel_v[pos] = (slot_of_dst[dst[e_c]] - win_c * 128).astype(F32)
        srl_rows[pos] = srl_bf[e_c]
        # dummy pads: repeat a valid row (idx stays 0 where no edge before;
        # fill window pads with the window's first real row for locality)
        for w in range(cfg.NWIN):
            if cnt[w] < cfg.C_WIN:
                fill = row[np.searchsorted(win_c, w)] if cnt[w] > 0 else 0
                idx[w * cfg.C_WIN + cnt[w]:(w + 1) * cfg.C_WIN] = fill

        srlT_c = np.ascontiguousarray(srl_rows.T)

        # dstrel blocks: per (window, tile-in-window) columns
        dstrel_blk = np.full((128, cfg.NBLK), -1.0, F32)
        slots_v = dstrel_v.reshape(cfg.NTILES, 128).T   # [128, tile]
        for w in range(cfg.NWIN):
            t0, t1 = cfg.win_t0[w], cfg.win_t1[w]
            b0 = int(cfg.win_blk0[w])
            base_shift = np.zeros(t1 - t0, F32)
            # dstrel_v holds slot - win*128 for the edge's own window; for a
            # straddle tile the neighbor window's edges carry their own
            # offset.  Rebase everything to window w:
            for j, t in enumerate(range(t0, t1)):
                col = slots_v[:, t].copy()
                # which window does each slot position belong to?
                slot_ids = t * 128 + np.arange(128)
                w_of_slot = slot_ids // cfg.C_WIN
                valid = col >= 0
                rb = col + (w_of_slot - w) * 128.0
                rb[~valid] = -1.0
                dstrel_blk[:, b0 + j] = rb
        dstrel_c = dstrel_blk.astype(BF16)

        def wrap_idx(arr):  # [SLOTS] -> [128, SLOTS//16]
            wr = arr.reshape(cfg.SLOTS // 16, 16).T
            return np.ascontiguousarray(np.tile(wr, (8, 1)))

        hsl = h_bf[c * cfg.NSH:(c + 1) * cfg.NSH]
        hT_c = np.ascontiguousarray(
            hsl.T.reshape(cfg.KC, 128, cfg.NSH).transpose(1, 0, 2))

        in_maps.append({
            "hT": hT_c,
            "WfcT": WfcT_r,
            "WfeT": WfeT_r,
            "IotaM": IotaM,
            "srlT": srlT_c,
            "dstrel": dstrel_c,
            "idxT": wrap_idx(idx),
        })
    return in_maps, slot_of_dst


def required_c_win(cfg: Cfg, src, dst):
    src = np.asarray(src, np.int64)
    dst = np.asarray(dst, np.int64)
    pair_of_edge = src // cfg.PAIR_ROWS
    slot_of_dst = _greedy_windows(cfg, dst, pair_of_edge)
    win_of_edge = slot_of_dst[dst] // 128
    # worst core count: ceil(pair-window count / 2)
    key = pair_of_edge * cfg.NWIN + win_of_edge
    counts = np.bincount(key, minlength=cfg.NPAIRS * cfg.NWIN)
    need = int(-(-counts.max() // 2))
    return -(-need // 16) * 16


# --------------------------------------------------------------------------
# entry point
# --------------------------------------------------------------------------

_CACHE = {}


def _get_graph(cfg: Cfg):
    key = (cfg.NCORES, cfg.NA_PAD, cfg.NP, cfg.C_WIN, cfg.CHUNK, cfg.SP)
    if key not in _CACHE:
        _CACHE[key] = build_graph(cfg)
    return _CACHE[key]


def kernel(h, srl_emb, src, dst, W_fc, W_feat, W_attn, _trace=False,
           _tmpdir=None):
    src = np.asarray(src)
    dst = np.asarray(dst)
    cfg = Cfg()
    need = required_c_win(cfg, src, dst)
    if need != cfg.C_WIN:
        cfg = Cfg(c_win=need)
    nc = _get_graph(cfg)
    in_maps, slot_of_dst = host_prep(
        cfg, np.asarray(h), np.asarray(srl_emb), src, dst,
        np.asarray(W_fc), np.asarray(W_feat), np.asarray(W_attn))
    res = run_bass_kernel_spmd(nc, in_maps, core_ids=list(range(cfg.NCORES)),
                               trace=_trace, tmpdir=_tmpdir)
    # reassemble: segment s, core c shard covers slots s*2048 + c*256 + [0,256)
    out_slots = np.empty((cfg.SLOT_ROWS, cfg.H, cfg.D), F32)
    NSEG = 2
    SPS = cfg.SLOT_ROWS // NSEG
    SHR = SPS // cfg.NCORES
    for c in range(cfg.NCORES):
        shard = np.asarray(res.results[c]["out"], F32)  # [1280, 256] d-major
        shard = shard.reshape(NSEG, SHR, cfg.D, cfg.H).transpose(0, 1, 3, 2)
        for s in range(NSEG):
            out_slots[s * SPS + c * SHR:s * SPS + (c + 1) * SHR] = shard[s]
    out = out_slots[slot_of_dst]
    if _trace:
        kernel._last_results = res
    return out


# revision 38
# speedup vs baseline: 1.3014x; 1.0072x over previous
"""Trainium2 distributed Bass kernel for the APGAT layer (gnn_message_passing).

v2 strategy (8 NeuronCores, 4 HBM pairs):
  - Nodes are sharded over cores (6272/core); cores (2k, 2k+1) share an HBM
    domain, so each pair holds a PAIR-LOCAL z table of 12544 rows in Shared
    DRAM (written with a rank-dependent dynamic-offset DMA; a tiny pair
    AllReduce acts as the readiness barrier).  12544 < 32768 so gather
    indices fit int16 with a SINGLE table - no A/B split.
  - Edges are assigned to the pair that owns their src node (gathers are
    always pair-local; no z AllGather at all), split evenly between the two
    cores of the pair.  Each core processes ~50k edges spanning ALL dst
    nodes; per-dst partial sums are combined at the end with two pipelined
    8-core ReduceScatters over the 10240 dst slots.
  - dst nodes are relabeled on the host (greedy, degree-balanced) into 80
    windows of 125 nodes so every (core, window) has <= C_WIN edges.
    Edge stream = 80 windows x C_WIN slots, padded with dummy (valid) idx
    rows that dstrel = -1 masks out of the one-hot.
  - Per 1792-edge chunk: SWDGE dma_gather of [z | s_src] rows (768 B),
    s_feat via PE (srl stationary), scores -> exp, msg = ex (x) z in the
    DVE 2x mode (d-major layout).  Per window: one-hot is_equal + PE
    segment-sum matmuls into PSUM, evicted bf16 to an SBUF accumulator.
  - Softmax max-subtraction is skipped (scores are O(7); validated vs the
    reference, gate is 2e-2).
"""

import sys

sys.path.insert(0, "/opt/trn_rl_repo")

import numpy as np
import ml_dtypes

import concourse.bass as bass
import concourse.bacc as bacc
import concourse.mybir as mybir
import concourse.tile as tile
from concourse.bass import ds
from concourse.tile import add_dep_helper
from concourse.bass_utils import run_bass_kernel_spmd

BF16 = ml_dtypes.bfloat16
F8 = ml_dtypes.float8_e4m3fn
F32 = np.float32
AF = mybir.ActivationFunctionType
ALU = mybir.AluOpType

N_SWDGE_Q = 8


class Cfg:
    def __init__(self, ncores=8, na=50000, np_=10000, e=400000, in_dim=512,
                 feat=128, h=8, d=32, c_win=640, chunk=640, single_packet=True):
        self.NCORES = ncores
        self.NPAIRS = ncores // 2
        self.NA = na
        self.NP = np_
        self.E = e
        self.IN_DIM = in_dim
        self.KC = in_dim // 128
        self.FEAT = feat
        self.H = h
        self.D = d
        self.HD = h * d                    # 256
        self.HDE = self.HD + h             # 264: [z | s_src]
        self.RW = 384                      # bf16 row width -> 768 B rows
        nsh = -(-na // ncores)
        self.NSH = -(-nsh // 128) * 128    # nodes per core (6272)
        self.NA_PAD = self.NSH * ncores
        self.NT = self.NSH // 128          # node tiles per core (49)
        self.PAIR_ROWS = 2 * self.NSH      # 12544 (< 32768: int16 idx)
        assert self.PAIR_ROWS <= 32768
        # dst windows: 80 windows x 125 nodes (128 slots each)
        self.NWIN = 80
        self.NODES_PER_WIN = np_ // self.NWIN    # 125
        self.SLOT_ROWS = self.NWIN * 128         # 10240 dst slots
        self.WIN_PER_CORE = self.NWIN // ncores  # 10
        # per (core, window) edge capacity
        assert c_win % 16 == 0
        self.C_WIN = c_win
        self.CHUNK = chunk                 # gather chunk (multiple of 128)
        assert chunk % 128 == 0
        self.SP = single_packet
        slots = self.NWIN * c_win
        self.NCHUNK = -(-slots // chunk)
        self.SLOTS = self.NCHUNK * chunk   # padded stream length
        self.TPC = chunk // 128            # tiles per chunk
        self.NTILES = self.SLOTS // 128
        # per-window tile spans (static given C_WIN)
        self.win_t0 = [(c_win * w) // 128 for w in range(self.NWIN)]
        self.win_t1 = [-(-(c_win * (w + 1)) // 128) for w in range(self.NWIN)]
        self.win_blk0 = np.cumsum([0] + [t1 - t0 for t0, t1 in
                                         zip(self.win_t0, self.win_t1)])
        self.NBLK = int(self.win_blk0[-1])


def build_graph(cfg: Cfg):
    nc = bacc.Bacc("TRN2", target_bir_lowering=False, debug=False,
                   num_devices=cfg.NCORES, num_swdge_queues=N_SWDGE_Q)
    bf = mybir.dt.bfloat16
    f8 = mybir.dt.float8e4
    f32 = mybir.dt.float32
    i16 = mybir.dt.int16
    u8 = mybir.dt.uint8

    # ---- kernel I/O ----
    hT = nc.dram_tensor("hT", [128, cfg.KC, cfg.NSH], bf, kind="ExternalInput")
    WfcT = nc.dram_tensor("WfcT", [128, cfg.KC, cfg.HDE], bf, kind="ExternalInput")
    WfeT = nc.dram_tensor("WfeT", [cfg.FEAT, cfg.H], bf, kind="ExternalInput")
    IotaM = nc.dram_tensor("IotaM", [128, 128], bf, kind="ExternalInput")
    srlT = nc.dram_tensor("srlT", [cfg.FEAT, cfg.SLOTS], bf, kind="ExternalInput")
    dstrel = nc.dram_tensor("dstrel", [128, cfg.NBLK], bf, kind="ExternalInput")
    idxT = nc.dram_tensor("idxT", [128, cfg.SLOTS // 16], i16, kind="ExternalInput")
    out_ext = nc.dram_tensor("out", [2 * cfg.WIN_PER_CORE // 2 * 128, cfg.HD],
                             f32, kind="ExternalOutput")   # [1280, 256]

    pair_groups = [[2 * p, 2 * p + 1] for p in range(cfg.NPAIRS)]
    all_group = [list(range(cfg.NCORES))]
    NSEG = 2                               # pipelined ReduceScatter halves
    WPS = cfg.NWIN // NSEG                 # windows per segment (16)
    SPS = WPS * 128                        # acc rows per segment (2048)
    SHR = SPS // cfg.NCORES                # shard rows per core (256)
    SHT = SHR // 128                       # shard row tiles (2)

    with tile.TileContext(nc) as tc:
        with (
            tc.tile_pool(name="dram", bufs=1, space="DRAM") as dram,
            tc.tile_pool(name="consts", bufs=1) as consts,
            tc.tile_pool(name="psum_sf", bufs=2, space="PSUM") as psum_sf,
            tc.tile_pool(name="psum_acc", bufs=2, space="PSUM") as psum_acc,
            tc.tile_pool(name="srl", bufs=4) as srl_pool,
            tc.tile_pool(name="zg", bufs=7) as zg_pool,
            tc.tile_pool(name="msg", bufs=6) as msg_pool,
            tc.tile_pool(name="oh", bufs=3) as oh_pool,
            tc.tile_pool(name="small", bufs=3) as small,
            tc.tile_pool(name="accsb", bufs=1) as accsb,
            tc.tile_pool(name="fin", bufs=2) as fin,
        ):
            # ---- shared/DRAM scratch ----
            z_pair = dram.tile([cfg.PAIR_ROWS, cfg.RW], bf, addr_space="Shared")
            bar_in = dram.tile([1, 16], bf)
            bar_out = dram.tile([1, 16], bf)
            acc_dram = [dram.tile([SPS, cfg.HDE], bf, name=f"accd{i}")
                        for i in range(NSEG)]
            rs_out = [dram.tile([SHR, cfg.HDE], bf, name=f"rs{i}")
                      for i in range(NSEG)]

            # ---- constants ----
            wfe_sb = consts.tile([cfg.FEAT, cfg.H], bf)
            nc.sync.dma_start(wfe_sb[:], WfeT[:])
            iota_sb = consts.tile([128, 128], bf)
            nc.sync.dma_start(iota_sb[:], IotaM[:])
            idx_sb = consts.tile([128, cfg.SLOTS // 16], i16)
            nc.sync.dma_start(idx_sb[:], idxT[:])
            dst_sb = consts.tile([128, cfg.NBLK], bf)
            nc.sync.dma_start(dst_sb[:], dstrel[:])

            # rank within the pair (0/1) for the z-table write offset
            rank1 = nc.sync.partition_id() % 2
            row_off = rank1 * cfg.NSH

            # ---- phase A: z = [h @ Wfc | s_src] -> pair-shared table ----
            with (
                tc.tile_pool(name="zph_h", bufs=2) as zph_h,
                tc.tile_pool(name="zph_w", bufs=1) as zph_w,
                tc.tile_pool(name="zph_s", bufs=1) as zph_s,
                tc.tile_pool(name="psum_z", bufs=4, space="PSUM") as psum_z,
            ):
                wfc_sb = zph_w.tile([128, cfg.KC, cfg.HDE], bf)
                nc.sync.dma_start(wfc_sb[:], WfcT[:])
                zstage = zph_s.tile([128, cfg.NT, cfg.HDE], bf)
                bounds = [0, 13, 25, 37, cfg.NT]
                for t0, t1 in zip(bounds[:-1], bounds[1:]):
                    nt = t1 - t0
                    hT_sb = zph_h.tile([128, cfg.KC, 13 * 128], bf,
                                       name=f"hT{t0}", tag="hT")
                    nc.sync.dma_start(hT_sb[:, :, 0:nt * 128],
                                      hT[:, :, t0 * 128:t1 * 128])
                    for i in range(nt):
                        pz = psum_z.tile([128, cfg.HDE], f32,
                                         name=f"pz{t0 + i}", tag="pz")
                        for c in range(cfg.KC):
                            nc.tensor.matmul(
                                pz[:],
                                hT_sb[:, c, i * 128:(i + 1) * 128],
                                wfc_sb[:, c, :],
                                start=(c == 0), stop=(c == cfg.KC - 1),
                            )
                        nc.scalar.copy(zstage[:, t0 + i, :], pz[:])
                z_write = nc.sync.dma_start(
                    z_pair[:][ds(row_off, cfg.NSH), 0:cfg.HDE]
                        .rearrange("(t p) r -> p t r", p=128),
                    zstage[:])

            # barrier: pair AllReduce; completes only after both pair cores
            # have finished their z-table writes
            nc.sync.dma_start(bar_in[:], iota_sb[0:1, 0:16])
            bar = nc.gpsimd.collective_compute(
                "AllReduce", ALU.add, ins=[bar_in[:].opt()],
                outs=[bar_out[:].opt()], replica_groups=pair_groups)
            add_dep_helper(bar.ins, z_write.ins, reason="barrier after z write")

            # ---- phase B ----
            # windows grouped by the chunk that completes them
            win_by_chunk = {}
            for w in range(cfg.NWIN):
                lc = (cfg.win_t1[w] - 1) // cfg.TPC
                win_by_chunk.setdefault(lc, []).append(w)

            msg_tiles = {}    # global tile idx -> (msg tile, local idx)
            evicts = []
            rs_pending = []
            acc_sb = [accsb.tile([128, WPS, cfg.HDE], bf, name=f"acc{i}")
                      for i in range(NSEG)]

            # prefetch: first srl chunks + one-hot blocks run during phase A
            srl_pre = {}
            for c in range(3):
                srl_sb = srl_pool.tile([cfg.FEAT, cfg.CHUNK], bf,
                                       name=f"srl{c}", tag="srl")
                nc.scalar.dma_start(
                    srl_sb[:], srlT[:, c * cfg.CHUNK:(c + 1) * cfg.CHUNK])
                srl_pre[c] = srl_sb

            oh_pre = {}

            def build_oh(w):
                t0, t1 = cfg.win_t0[w], cfg.win_t1[w]
                ntw = t1 - t0
                blk0 = int(cfg.win_blk0[w])
                oh = oh_pool.tile([128, ntw, 128], bf, name=f"oh{w}", tag="oh")
                nc.vector.tensor_tensor(
                    oh[:],
                    iota_sb[:].unsqueeze(1).broadcast_to([128, ntw, 128]),
                    dst_sb[:, blk0:blk0 + ntw]
                        .unsqueeze(2).broadcast_to([128, ntw, 128]),
                    ALU.is_equal,
                )
                return oh

            for w in range(3):
                oh_pre[w] = build_oh(w)

            def emit_window(w):
                t0, t1 = cfg.win_t0[w], cfg.win_t1[w]
                ntw = t1 - t0
                oh = oh_pre.pop(w, None)
                if oh is None:
                    oh = build_oh(w)
                pacc = psum_acc.tile([128, cfg.HDE], f32,
                                     name=f"pacc{w}", tag="pacc")
                for j, t in enumerate(range(t0, t1)):
                    mt, li = msg_tiles[t]
                    nc.tensor.matmul(
                        pacc[:],
                        oh[:, j, :],
                        mt[:, li, :],
                        start=(j == 0), stop=(j == ntw - 1),
                    )
                seg, wl = w // WPS, w % WPS
                cp = nc.scalar.copy(acc_sb[seg][:, wl, :], pacc[:])
                evicts.append(cp.ins)

            def emit_rs(seg):
                nc.sync.dma_start(
                    acc_dram[seg][:]
                        .rearrange("(w p) c -> p w c", p=128),
                    acc_sb[seg][:])
                nc.gpsimd.collective_compute(
                    "ReduceScatter", ALU.add,
                    ins=[acc_dram[seg][:].opt()],
                    outs=[rs_out[seg][:].opt()],
                    replica_groups=all_group)

            for c in range(cfg.NCHUNK):
                zg = zg_pool.tile([128, cfg.TPC, cfg.RW], bf,
                                  name=f"zg{c}", tag="zg")
                g = nc.gpsimd.dma_gather(
                    zg[:], z_pair[:],
                    idx_sb[:, c * (cfg.CHUNK // 16):(c + 1) * (cfg.CHUNK // 16)],
                    cfg.CHUNK, cfg.CHUNK, cfg.RW,
                    single_packet=cfg.SP,
                    queue_num=c % N_SWDGE_Q,
                )
                add_dep_helper(g.ins, bar.ins, reason="gather after barrier")

                srl_sb = srl_pre.pop(c, None)
                if srl_sb is None:
                    srl_sb = srl_pool.tile([cfg.FEAT, cfg.CHUNK], bf,
                                           name=f"srl{c}", tag="srl")
                    nc.scalar.dma_start(
                        srl_sb[:], srlT[:, c * cfg.CHUNK:(c + 1) * cfg.CHUNK])

                # scores: copy s_src into PSUM, matmul-accumulate s_feat on
                # top (start=False), then lrelu+exp on the Act engine
                psf = psum_sf.tile([128, cfg.TPC, cfg.H], f32,
                                   name=f"psf{c}", tag="psf")
                nc.scalar.copy(psf[:], zg[:, :, cfg.HD:cfg.HDE])
                for t in range(cfg.TPC):
                    nc.tensor.matmul(
                        psf[:, t, :],
                        srl_sb[:, t * 128:(t + 1) * 128],
                        wfe_sb[:],
                        start=False, stop=True,
                    )
                sfs = small.tile([128, cfg.TPC, cfg.H], f32,
                                 name=f"sfs{c}", tag="sfs")
                nc.scalar.copy(sfs[:], psf[:])
                lr = small.tile([128, cfg.TPC, cfg.H], f32,
                                name=f"lr{c}", tag="lr")
                nc.vector.scalar_tensor_tensor(
                    lr[:], sfs[:], 0.01, sfs[:], ALU.mult, ALU.max)

                msg = msg_pool.tile([128, cfg.TPC, cfg.HDE], bf,
                                    name=f"msg{c}", tag="msg")
                nc.scalar.activation(msg[:, :, cfg.HD:], lr[:], AF.Exp)
                nc.vector.tensor_tensor(
                    msg[:, :, 0:cfg.HD].rearrange("p t (d h) -> p t d h", h=cfg.H),
                    zg[:, :, 0:cfg.HD].rearrange("p t (d h) -> p t d h", h=cfg.H),
                    msg[:, :, cfg.HD:].unsqueeze(2)
                        .broadcast_to([128, cfg.TPC, cfg.D, cfg.H]),
                    ALU.mult,
                )
                for t in range(cfg.TPC):
                    msg_tiles[c * cfg.TPC + t] = (msg, t)

                for w in win_by_chunk.get(c, []):
                    emit_window(w)
                    if w % WPS == WPS - 1:
                        rs_pending.append((c + 2, w // WPS))
                while rs_pending and rs_pending[0][0] <= c:
                    emit_rs(rs_pending.pop(0)[1])
            while rs_pending:
                emit_rs(rs_pending.pop(0)[1])

            # ---- finalize: out = msg_tot / max(den, eps) per RS shard ----
            for seg in range(NSEG):
                tot = fin.tile([128, SHT, cfg.HDE], bf,
                               name=f"tot{seg}", tag="tot")
                tl = nc.sync.dma_start(
                    tot[:],
                    rs_out[seg][:].rearrange("(w p) c -> p w c", p=128))
                add_dep_helper(tl.ins, evicts[-1],
                               reason="finalize after last evict")
                den = fin.tile([128, SHT, cfg.H], f32,
                               name=f"den{seg}", tag="den")
                nc.scalar.activation(den[:], tot[:, :, cfg.HD:cfg.HDE],
                                     AF.Copy, bias=1e-9)
                rec = fin.tile([128, SHT, cfg.H], f32,
                               name=f"rec{seg}", tag="rec")
                nc.vector.reciprocal(rec[:], den[:])
                ow = fin.tile([128, SHT, cfg.HD], f32,
                              name=f"ow{seg}", tag="ow")
                nc.vector.tensor_tensor(
                    ow[:].rearrange("p w (d h) -> p w d h", h=cfg.H),
                    tot[:, :, 0:cfg.HD].rearrange("p w (d h) -> p w d h", h=cfg.H),
                    rec[:].unsqueeze(2)
                        .broadcast_to([128, SHT, cfg.D, cfg.H]),
                    ALU.mult,
                )
                nc.sync.dma_start(
                    out_ext[seg * SHR:(seg + 1) * SHR, :]
                        .rearrange("(w p) c -> p w c", p=128),
                    ow[:])

    nc.compile()
    return nc


# --------------------------------------------------------------------------
# host-side preprocessing
# --------------------------------------------------------------------------

def _greedy_windows(cfg: Cfg, dst, pair_of_edge):
    """Assign dst nodes to 80 windows of 125, balancing per-pair edge load.
    Returns slot_of_dst [NP] (window*128 + position)."""
    NW = cfg.NWIN
    # per (dst, pair) degree
    deg = np.zeros((cfg.NP, cfg.NPAIRS), np.int64)
    np.add.at(deg, (dst, pair_of_edge), 1)
    tot = deg.sum(1)
    order = np.argsort(-tot, kind="stable")
    load = np.zeros((NW, cfg.NPAIRS), np.int64)
    count = np.zeros(NW, np.int64)
    win_of = np.empty(cfg.NP, np.int64)
    pos_of = np.empty(cfg.NP, np.int64)
    for d in order:
        cand = load + deg[d][None, :]
        score = cand.max(1) * 1000 + cand.sum(1)
        score[count >= cfg.NODES_PER_WIN] = np.iinfo(np.int64).max
        w = int(np.argmin(score))
        win_of[d] = w
        pos_of[d] = count[w]
        count[w] += 1
        load[w] += deg[d]
    assert (count == cfg.NODES_PER_WIN).all()
    return win_of * 128 + pos_of


def host_prep(cfg: Cfg, h, srl_emb, src, dst, W_fc, W_feat, W_attn):
    H, D = cfg.H, cfg.D

    a = np.asarray(W_attn, F32)[0]
    a_src, a_feat = a[:D], a[2 * D:3 * D]
    W_fc = np.asarray(W_fc, F32)
    Wf_eff = (np.asarray(W_feat, F32).reshape(H, D, cfg.FEAT)
              * a_feat[None, :, None]).sum(1)
    Wz_eff = (W_fc.reshape(H, D, cfg.IN_DIM) * a_src[None, :, None]).sum(1)

    # d-major column order for z: col j <-> (h=j%8, d=j//8)
    perm = np.array([(j % H) * D + j // H for j in range(cfg.HD)], np.int64)
    Wfull = np.concatenate([W_fc.T[:, perm], Wz_eff.T], axis=1)  # [IN_DIM, HDE]
    WfcT_r = np.ascontiguousarray(
        Wfull.reshape(cfg.KC, 128, cfg.HDE).transpose(1, 0, 2)).astype(BF16)
    WfeT_r = np.ascontiguousarray(Wf_eff.T).astype(BF16)
    IotaM = np.tile(np.arange(128, dtype=F32)[None, :], (128, 1)).astype(BF16)

    h_bf = np.zeros((cfg.NA_PAD, cfg.IN_DIM), BF16)
    h_bf[:cfg.NA] = np.asarray(h, F32).astype(BF16)
    srl_bf = np.asarray(srl_emb, F32).astype(BF16)

    src = np.asarray(src, np.int64)
    dst = np.asarray(dst, np.int64)
    pair_of_edge = src // cfg.PAIR_ROWS          # src in padded node space
    slot_of_dst = _greedy_windows(cfg, dst, pair_of_edge)
    win_of_edge = slot_of_dst[dst] // 128

    # core assignment: within (pair, window), alternate between pair cores
    order = np.lexsort((src, win_of_edge, pair_of_edge))
    e_s = order
    pair_s = pair_of_edge[e_s]
    win_s = win_of_edge[e_s]
    key = pair_s * cfg.NWIN + win_s
    # rank within each (pair, window) group
    grp_start = np.r_[True, key[1:] != key[:-1]]
    gidx = np.arange(len(e_s)) - np.maximum.accumulate(
        np.where(grp_start, np.arange(len(e_s)), 0))
    core_s = pair_s * 2 + (gidx % 2)

    in_maps = []
    for c in range(cfg.NCORES):
        sel = core_s == c
        e_c = e_s[sel]                     # sorted by (window, src)
        win_c = win_s[sel]
        cnt = np.bincount(win_c, minlength=cfg.NWIN)
        assert cnt.max() <= cfg.C_WIN, f"C_WIN too small: {cnt.max()}"

        idx = np.zeros(cfg.SLOTS, np.int16)
        dstrel_v = np.full(cfg.SLOTS, -1.0, F32)
        srl_rows = np.zeros((cfg.SLOTS, cfg.FEAT), BF16)
        pos = win_c * cfg.C_WIN + (
            np.arange(len(e_c)) - np.r_[0, np.cumsum(cnt)][win_c])
        row = (src[e_c] - (c // 2) * cfg.PAIR_ROWS).astype(np.int16)
        idx[pos] = row
        dstrel_v[pos] = (slot_of_dst[dst[e_c]] - win_c * 128).astype(F32)
        srl_rows[pos] = srl_bf[e_c]
        # dummy pads: repeat a valid row (idx stays 0 where no edge before;
        # fill window pads with the window's first real row for locality)
        for w in range(cfg.NWIN):
            if cnt[w] < cfg.C_WIN:
                fill = row[np.searchsorted(win_c, w)] if cnt[w] > 0 else 0
                idx[w * cfg.C_WIN + cnt[w]:(w + 1) * cfg.C_WIN] = fill

        srlT_c = np.ascontiguousarray(srl_rows.T)

        # dstrel blocks: per (window, tile-in-window) columns
        dstrel_blk = np.full((128, cfg.NBLK), -1.0, F32)
        slots_v = dstrel_v.reshape(cfg.NTILES, 128).T   # [128, tile]
        for w in range(cfg.NWIN):
            t0, t1 = cfg.win_t0[w], cfg.win_t1[w]
            b0 = int(cfg.win_blk0[w])
            base_shift = np.zeros(t1 - t0, F32)
            # dstrel_v holds slot - win*128 for the edge's own window; for a
            # straddle tile the neighbor window's edges carry their own
            # offset.  Rebase everything to window w:
            for j, t in enumerate(range(t0, t1)):
                col = slots_v[:, t].copy()
                # which window does each slot position belong to?
                slot_ids = t * 128 + np.arange(128)
                w_of_slot = slot_ids // cfg.C_WIN
                valid = col >= 0
                rb = col + (w_of_slot - w) * 128.0
                rb[~valid] = -1.0
                dstrel_blk[:, b0 + j] = rb
        dstrel_c = dstrel_blk.astype(BF16)

        def wrap_idx(arr):  # [SLOTS] -> [128, SLOTS//16]
            wr = arr.reshape(cfg.SLOTS // 16, 16).T
            return np.ascontiguousarray(np.tile(wr, (8, 1)))

        hsl = h_bf[c * cfg.NSH:(c + 1) * cfg.NSH]
        hT_c = np.ascontiguousarray(
            hsl.T.reshape(cfg.KC, 128, cfg.NSH).transpose(1, 0, 2))

        in_maps.append({
            "hT": hT_c,
            "WfcT": WfcT_r,
            "WfeT": WfeT_r,
            "IotaM": IotaM,
            "srlT": srlT_c,
            "dstrel": dstrel_c,
            "idxT": wrap_idx(idx),
        })
    return in_maps, slot_of_dst


def required_c_win(cfg: Cfg, src, dst):
    src = np.asarray(src, np.int64)
    dst = np.asarray(dst, np.int64)
    pair_of_edge = src // cfg.PAIR_ROWS
    slot_of_dst = _greedy_windows(cfg, dst, pair_of_edge)
    win_of_edge = slot_of_dst[dst] // 128
    # worst core count: ceil(pair-window count / 2)
    key = pair_of_edge * cfg.NWIN + win_of_edge
    counts = np.bincount(key, minlength=cfg.NPAIRS * cfg.NWIN)
    need = int(-(-counts.max() // 2))
    return -(-need // 16) * 16


# --------------------------------------------------------------------------
# entry point
# --------------------------------------------------------------------------

_CACHE = {}


def _get_graph(cfg: Cfg):
    key = (cfg.NCORES, cfg.NA_PAD, cfg.NP, cfg.C_WIN, cfg.CHUNK, cfg.SP)
    if key not in _CACHE:
        _CACHE[key] = build_graph(cfg)
    return _CACHE[key]


def kernel(h, srl_emb, src, dst, W_fc, W_feat, W_attn, _trace=False,
           _tmpdir=None):
    src = np.asarray(src)
    dst = np.asarray(dst)
    cfg = Cfg()
    need = required_c_win(cfg, src, dst)
    if need != cfg.C_WIN:
        cfg = Cfg(c_win=need)
    nc = _get_graph(cfg)
    in_maps, slot_of_dst = host_prep(
        cfg, np.asarray(h), np.asarray(srl_emb), src, dst,
        np.asarray(W_fc), np.asarray(W_feat), np.asarray(W_attn))
    res = run_bass_kernel_spmd(nc, in_maps, core_ids=list(range(cfg.NCORES)),
                               trace=_trace, tmpdir=_tmpdir)
    # reassemble: segment s, core c shard covers slots s*2048 + c*256 + [0,256)
    out_slots = np.empty((cfg.SLOT_ROWS, cfg.H, cfg.D), F32)
    NSEG = 2
    SPS = cfg.SLOT_ROWS // NSEG
    SHR = SPS // cfg.NCORES
    for c in range(cfg.NCORES):
        shard = np.asarray(res.results[c]["out"], F32)  # [1280, 256] d-major
        shard = shard.reshape(NSEG, SHR, cfg.D, cfg.H).transpose(0, 1, 3, 2)
        for s in range(NSEG):
            out_slots[s * SPS + c * SHR:s * SPS + (c + 1) * SHR] = shard[s]
    out = out_slots[slot_of_dst]
    if _trace:
        kernel._last_results = res
    return out


# revision 39
# speedup vs baseline: 1.3066x; 1.0040x over previous
"""Trainium2 distributed Bass kernel for the APGAT layer (gnn_message_passing).

v2 strategy (8 NeuronCores, 4 HBM pairs):
  - Nodes are sharded over cores (6272/core); cores (2k, 2k+1) share an HBM
    domain, so each pair holds a PAIR-LOCAL z table of 12544 rows in Shared
    DRAM (written with a rank-dependent dynamic-offset DMA; a tiny pair
    AllReduce acts as the readiness barrier).  12544 < 32768 so gather
    indices fit int16 with a SINGLE table - no A/B split.
  - Edges are assigned to the pair that owns their src node (gathers are
    always pair-local; no z AllGather at all), split evenly between the two
    cores of the pair.  Each core processes ~50k edges spanning ALL dst
    nodes; per-dst partial sums are combined at the end with two pipelined
    8-core ReduceScatters over the 10240 dst slots.
  - dst nodes are relabeled on the host (greedy, degree-balanced) into 80
    windows of 125 nodes so every (core, window) has <= C_WIN edges.
    Edge stream = 80 windows x C_WIN slots, padded with dummy (valid) idx
    rows that dstrel = -1 masks out of the one-hot.
  - Per 1792-edge chunk: SWDGE dma_gather of [z | s_src] rows (768 B),
    s_feat via PE (srl stationary), scores -> exp, msg = ex (x) z in the
    DVE 2x mode (d-major layout).  Per window: one-hot is_equal + PE
    segment-sum matmuls into PSUM, evicted bf16 to an SBUF accumulator.
  - Softmax max-subtraction is skipped (scores are O(7); validated vs the
    reference, gate is 2e-2).
"""

import sys

sys.path.insert(0, "/opt/trn_rl_repo")

import numpy as np
import ml_dtypes

import concourse.bass as bass
import concourse.bacc as bacc
import concourse.mybir as mybir
import concourse.tile as tile
from concourse.bass import ds
from concourse.tile import add_dep_helper
from concourse.bass_utils import run_bass_kernel_spmd

BF16 = ml_dtypes.bfloat16
F8 = ml_dtypes.float8_e4m3fn
F32 = np.float32
AF = mybir.ActivationFunctionType
ALU = mybir.AluOpType

N_SWDGE_Q = 8


class Cfg:
    def __init__(self, ncores=8, na=50000, np_=10000, e=400000, in_dim=512,
                 feat=128, h=8, d=32, c_win=640, chunk=640, single_packet=True):
        self.NCORES = ncores
        self.NPAIRS = ncores // 2
        self.NA = na
        self.NP = np_
        self.E = e
        self.IN_DIM = in_dim
        self.KC = in_dim // 128
        self.FEAT = feat
        self.H = h
        self.D = d
        self.HD = h * d                    # 256
        self.HDE = self.HD + h             # 264: [z | s_src]
        self.RW = 384                      # bf16 row width -> 768 B rows
        nsh = -(-na // ncores)
        self.NSH = -(-nsh // 128) * 128    # nodes per core (6272)
        self.NA_PAD = self.NSH * ncores
        self.NT = self.NSH // 128          # node tiles per core (49)
        self.PAIR_ROWS = 2 * self.NSH      # 12544 (< 32768: int16 idx)
        assert self.PAIR_ROWS <= 32768
        # dst windows: 80 windows x 125 nodes (128 slots each)
        self.NWIN = 80
        self.NODES_PER_WIN = np_ // self.NWIN    # 125
        self.SLOT_ROWS = self.NWIN * 128         # 10240 dst slots
        self.WIN_PER_CORE = self.NWIN // ncores  # 10
        # per (core, window) edge capacity
        assert c_win % 16 == 0
        self.C_WIN = c_win
        self.CHUNK = chunk                 # gather chunk (multiple of 128)
        assert chunk % 128 == 0
        self.SP = single_packet
        slots = self.NWIN * c_win
        self.NCHUNK = -(-slots // chunk)
        self.SLOTS = self.NCHUNK * chunk   # padded stream length
        self.TPC = chunk // 128            # tiles per chunk
        self.NTILES = self.SLOTS // 128
        # per-window tile spans (static given C_WIN)
        self.win_t0 = [(c_win * w) // 128 for w in range(self.NWIN)]
        self.win_t1 = [-(-(c_win * (w + 1)) // 128) for w in range(self.NWIN)]
        self.win_blk0 = np.cumsum([0] + [t1 - t0 for t0, t1 in
                                         zip(self.win_t0, self.win_t1)])
        self.NBLK = int(self.win_blk0[-1])


def build_graph(cfg: Cfg):
    nc = bacc.Bacc("TRN2", target_bir_lowering=False, debug=False,
                   num_devices=cfg.NCORES, num_swdge_queues=N_SWDGE_Q)
    bf = mybir.dt.bfloat16
    f8 = mybir.dt.float8e4
    f32 = mybir.dt.float32
    i16 = mybir.dt.int16
    u8 = mybir.dt.uint8

    # ---- kernel I/O ----
    hT = nc.dram_tensor("hT", [128, cfg.KC, cfg.NSH], bf, kind="ExternalInput")
    WfcT = nc.dram_tensor("WfcT", [128, cfg.KC, cfg.HDE], bf, kind="ExternalInput")
    WfeT = nc.dram_tensor("WfeT", [cfg.FEAT, cfg.H], bf, kind="ExternalInput")
    IotaM = nc.dram_tensor("IotaM", [128, 128], bf, kind="ExternalInput")
    srlT = nc.dram_tensor("srlT", [cfg.FEAT, cfg.SLOTS], bf, kind="ExternalInput")
    dstrel = nc.dram_tensor("dstrel", [128, cfg.NBLK], bf, kind="ExternalInput")
    idxT = nc.dram_tensor("idxT", [128, cfg.SLOTS // 16], i16, kind="ExternalInput")
    out_ext = nc.dram_tensor("out", [2 * cfg.WIN_PER_CORE // 2 * 128, cfg.HD],
                             f32, kind="ExternalOutput")   # [1280, 256]

    pair_groups = [[2 * p, 2 * p + 1] for p in range(cfg.NPAIRS)]
    all_group = [list(range(cfg.NCORES))]
    NSEG = 2                               # pipelined ReduceScatter halves
    WPS = cfg.NWIN // NSEG                 # windows per segment (16)
    SPS = WPS * 128                        # acc rows per segment (2048)
    SHR = SPS // cfg.NCORES                # shard rows per core (256)
    SHT = SHR // 128                       # shard row tiles (2)

    with tile.TileContext(nc) as tc:
        with (
            tc.tile_pool(name="dram", bufs=1, space="DRAM") as dram,
            tc.tile_pool(name="consts", bufs=1) as consts,
            tc.tile_pool(name="psum_sf", bufs=2, space="PSUM") as psum_sf,
            tc.tile_pool(name="psum_acc", bufs=2, space="PSUM") as psum_acc,
            tc.tile_pool(name="srl", bufs=4) as srl_pool,
            tc.tile_pool(name="zg", bufs=9) as zg_pool,
            tc.tile_pool(name="msg", bufs=7) as msg_pool,
            tc.tile_pool(name="oh", bufs=3) as oh_pool,
            tc.tile_pool(name="small", bufs=3) as small,
            tc.tile_pool(name="accsb", bufs=1) as accsb,
            tc.tile_pool(name="fin", bufs=2) as fin,
        ):
            # ---- shared/DRAM scratch ----
            z_pair = dram.tile([cfg.PAIR_ROWS, cfg.RW], bf, addr_space="Shared")
            bar_in = dram.tile([1, 16], bf)
            bar_out = dram.tile([1, 16], bf)
            acc_dram = [dram.tile([SPS, cfg.HDE], bf, name=f"accd{i}")
                        for i in range(NSEG)]
            rs_out = [dram.tile([SHR, cfg.HDE], bf, name=f"rs{i}")
                      for i in range(NSEG)]

            # ---- constants ----
            wfe_sb = consts.tile([cfg.FEAT, cfg.H], bf)
            nc.sync.dma_start(wfe_sb[:], WfeT[:])
            iota_sb = consts.tile([128, 128], bf)
            nc.sync.dma_start(iota_sb[:], IotaM[:])
            idx_sb = consts.tile([128, cfg.SLOTS // 16], i16)
            nc.sync.dma_start(idx_sb[:], idxT[:])
            dst_sb = consts.tile([128, cfg.NBLK], bf)
            nc.sync.dma_start(dst_sb[:], dstrel[:])

            # rank within the pair (0/1) for the z-table write offset
            rank1 = nc.sync.partition_id() % 2
            row_off = rank1 * cfg.NSH

            # ---- phase A: z = [h @ Wfc | s_src] -> pair-shared table ----
            with (
                tc.tile_pool(name="zph_h", bufs=2) as zph_h,
                tc.tile_pool(name="zph_w", bufs=1) as zph_w,
                tc.tile_pool(name="zph_s", bufs=1) as zph_s,
                tc.tile_pool(name="psum_z", bufs=4, space="PSUM") as psum_z,
            ):
                wfc_sb = zph_w.tile([128, cfg.KC, cfg.HDE], bf)
                nc.sync.dma_start(wfc_sb[:], WfcT[:])
                zstage = zph_s.tile([128, cfg.NT, cfg.HDE], bf)
                bounds = [0, 13, 25, 37, cfg.NT]
                for t0, t1 in zip(bounds[:-1], bounds[1:]):
                    nt = t1 - t0
                    hT_sb = zph_h.tile([128, cfg.KC, 13 * 128], bf,
                                       name=f"hT{t0}", tag="hT")
                    nc.sync.dma_start(hT_sb[:, :, 0:nt * 128],
                                      hT[:, :, t0 * 128:t1 * 128])
                    for i in range(nt):
                        pz = psum_z.tile([128, cfg.HDE], f32,
                                         name=f"pz{t0 + i}", tag="pz")
                        for c in range(cfg.KC):
                            nc.tensor.matmul(
                                pz[:],
                                hT_sb[:, c, i * 128:(i + 1) * 128],
                                wfc_sb[:, c, :],
                                start=(c == 0), stop=(c == cfg.KC - 1),
                            )
                        nc.scalar.copy(zstage[:, t0 + i, :], pz[:])
                z_write = nc.sync.dma_start(
                    z_pair[:][ds(row_off, cfg.NSH), 0:cfg.HDE]
                        .rearrange("(t p) r -> p t r", p=128),
                    zstage[:])

            # barrier: pair AllReduce; completes only after both pair cores
            # have finished their z-table writes
            nc.sync.dma_start(bar_in[:], iota_sb[0:1, 0:16])
            bar = nc.gpsimd.collective_compute(
                "AllReduce", ALU.add, ins=[bar_in[:].opt()],
                outs=[bar_out[:].opt()], replica_groups=pair_groups)
            add_dep_helper(bar.ins, z_write.ins, reason="barrier after z write")

            # ---- phase B ----
            # windows grouped by the chunk that completes them
            win_by_chunk = {}
            for w in range(cfg.NWIN):
                lc = (cfg.win_t1[w] - 1) // cfg.TPC
                win_by_chunk.setdefault(lc, []).append(w)

            msg_tiles = {}    # global tile idx -> (msg tile, local idx)
            evicts = []
            rs_pending = []
            acc_sb = [accsb.tile([128, WPS, cfg.HDE], bf, name=f"acc{i}")
                      for i in range(NSEG)]

            # prefetch: first srl chunks + one-hot blocks run during phase A
            srl_pre = {}
            for c in range(3):
                srl_sb = srl_pool.tile([cfg.FEAT, cfg.CHUNK], bf,
                                       name=f"srl{c}", tag="srl")
                nc.scalar.dma_start(
                    srl_sb[:], srlT[:, c * cfg.CHUNK:(c + 1) * cfg.CHUNK])
                srl_pre[c] = srl_sb

            oh_pre = {}

            def build_oh(w):
                t0, t1 = cfg.win_t0[w], cfg.win_t1[w]
                ntw = t1 - t0
                blk0 = int(cfg.win_blk0[w])
                oh = oh_pool.tile([128, ntw, 128], bf, name=f"oh{w}", tag="oh")
                nc.vector.tensor_tensor(
                    oh[:],
                    iota_sb[:].unsqueeze(1).broadcast_to([128, ntw, 128]),
                    dst_sb[:, blk0:blk0 + ntw]
                        .unsqueeze(2).broadcast_to([128, ntw, 128]),
                    ALU.is_equal,
                )
                return oh

            for w in range(3):
                oh_pre[w] = build_oh(w)

            def emit_window(w):
                t0, t1 = cfg.win_t0[w], cfg.win_t1[w]
                ntw = t1 - t0
                oh = oh_pre.pop(w, None)
                if oh is None:
                    oh = build_oh(w)
                pacc = psum_acc.tile([128, cfg.HDE], f32,
                                     name=f"pacc{w}", tag="pacc")
                for j, t in enumerate(range(t0, t1)):
                    mt, li = msg_tiles[t]
                    nc.tensor.matmul(
                        pacc[:],
                        oh[:, j, :],
                        mt[:, li, :],
                        start=(j == 0), stop=(j == ntw - 1),
                    )
                seg, wl = w // WPS, w % WPS
                cp = nc.scalar.copy(acc_sb[seg][:, wl, :], pacc[:])
                evicts.append(cp.ins)

            def emit_rs(seg):
                nc.sync.dma_start(
                    acc_dram[seg][:]
                        .rearrange("(w p) c -> p w c", p=128),
                    acc_sb[seg][:])
                nc.gpsimd.collective_compute(
                    "ReduceScatter", ALU.add,
                    ins=[acc_dram[seg][:].opt()],
                    outs=[rs_out[seg][:].opt()],
                    replica_groups=all_group)

            for c in range(cfg.NCHUNK):
                zg = zg_pool.tile([128, cfg.TPC, cfg.RW], bf,
                                  name=f"zg{c}", tag="zg")
                g = nc.gpsimd.dma_gather(
                    zg[:], z_pair[:],
                    idx_sb[:, c * (cfg.CHUNK // 16):(c + 1) * (cfg.CHUNK // 16)],
                    cfg.CHUNK, cfg.CHUNK, cfg.RW,
                    single_packet=cfg.SP,
                    queue_num=c % N_SWDGE_Q,
                )
                add_dep_helper(g.ins, bar.ins, reason="gather after barrier")

                srl_sb = srl_pre.pop(c, None)
                if srl_sb is None:
                    srl_sb = srl_pool.tile([cfg.FEAT, cfg.CHUNK], bf,
                                           name=f"srl{c}", tag="srl")
                    nc.scalar.dma_start(
                        srl_sb[:], srlT[:, c * cfg.CHUNK:(c + 1) * cfg.CHUNK])

                # scores: copy s_src into PSUM, matmul-accumulate s_feat on
                # top (start=False), then lrelu+exp on the Act engine
                psf = psum_sf.tile([128, cfg.TPC, cfg.H], f32,
                                   name=f"psf{c}", tag="psf")
                nc.scalar.copy(psf[:], zg[:, :, cfg.HD:cfg.HDE])
                for t in range(cfg.TPC):
                    nc.tensor.matmul(
                        psf[:, t, :],
                        srl_sb[:, t * 128:(t + 1) * 128],
                        wfe_sb[:],
                        start=False, stop=True,
                    )
                sfs = small.tile([128, cfg.TPC, cfg.H], f32,
                                 name=f"sfs{c}", tag="sfs")
                nc.scalar.copy(sfs[:], psf[:])
                lr = small.tile([128, cfg.TPC, cfg.H], f32,
                                name=f"lr{c}", tag="lr")
                nc.vector.scalar_tensor_tensor(
                    lr[:], sfs[:], 0.01, sfs[:], ALU.mult, ALU.max)

                msg = msg_pool.tile([128, cfg.TPC, cfg.HDE], bf,
                                    name=f"msg{c}", tag="msg")
                nc.scalar.activation(msg[:, :, cfg.HD:], lr[:], AF.Exp)
                nc.vector.tensor_tensor(
                    msg[:, :, 0:cfg.HD].rearrange("p t (d h) -> p t d h", h=cfg.H),
                    zg[:, :, 0:cfg.HD].rearrange("p t (d h) -> p t d h", h=cfg.H),
                    msg[:, :, cfg.HD:].unsqueeze(2)
                        .broadcast_to([128, cfg.TPC, cfg.D, cfg.H]),
                    ALU.mult,
                )
                for t in range(cfg.TPC):
                    msg_tiles[c * cfg.TPC + t] = (msg, t)

                for w in win_by_chunk.get(c, []):
                    emit_window(w)
                    if w % WPS == WPS - 1:
                        rs_pending.append((c + 2, w // WPS))
                while rs_pending and rs_pending[0][0] <= c:
                    emit_rs(rs_pending.pop(0)[1])
            while rs_pending:
                emit_rs(rs_pending.pop(0)[1])

            # ---- finalize: out = msg_tot / max(den, eps) per RS shard ----
            for seg in range(NSEG):
                tot = fin.tile([128, SHT, cfg.HDE], bf,
                               name=f"tot{seg}", tag="tot")
                tl = nc.sync.dma_start(
                    tot[:],
                    rs_out[seg][:].rearrange("(w p) c -> p w c", p=128))
                add_dep_helper(tl.ins, evicts[-1],
                               reason="finalize after last evict")
                den = fin.tile([128, SHT, cfg.H], f32,
                               name=f"den{seg}", tag="den")
                nc.scalar.activation(den[:], tot[:, :, cfg.HD:cfg.HDE],
                                     AF.Copy, bias=1e-9)
                rec = fin.tile([128, SHT, cfg.H], f32,
                               name=f"rec{seg}", tag="rec")
                nc.vector.reciprocal(rec[:], den[:])
                ow = fin.tile([128, SHT, cfg.HD], f32,
                              name=f"ow{seg}", tag="ow")
                nc.vector.tensor_tensor(
                    ow[:].rearrange("p w (d h) -> p w d h", h=cfg.H),
                    tot[:, :, 0:cfg.HD].rearrange("p w (d h) -> p w d h", h=cfg.H),
                    rec[:].unsqueeze(2)
                        .broadcast_to([128, SHT, cfg.D, cfg.H]),
                    ALU.mult,
                )
                nc.sync.dma_start(
                    out_ext[seg * SHR:(seg + 1) * SHR, :]
                        .rearrange("(w p) c -> p w c", p=128),
                    ow[:])

    nc.compile()
    return nc


# --------------------------------------------------------------------------
# host-side preprocessing
# --------------------------------------------------------------------------

def _greedy_windows(cfg: Cfg, dst, pair_of_edge):
    """Assign dst nodes to 80 windows of 125, balancing per-pair edge load.
    Returns slot_of_dst [NP] (window*128 + position)."""
    NW = cfg.NWIN
    # per (dst, pair) degree
    deg = np.zeros((cfg.NP, cfg.NPAIRS), np.int64)
    np.add.at(deg, (dst, pair_of_edge), 1)
    tot = deg.sum(1)
    order = np.argsort(-tot, kind="stable")
    load = np.zeros((NW, cfg.NPAIRS), np.int64)
    count = np.zeros(NW, np.int64)
    win_of = np.empty(cfg.NP, np.int64)
    pos_of = np.empty(cfg.NP, np.int64)
    for d in order:
        cand = load + deg[d][None, :]
        score = cand.max(1) * 1000 + cand.sum(1)
        score[count >= cfg.NODES_PER_WIN] = np.iinfo(np.int64).max
        w = int(np.argmin(score))
        win_of[d] = w
        pos_of[d] = count[w]
        count[w] += 1
        load[w] += deg[d]
    assert (count == cfg.NODES_PER_WIN).all()
    return win_of * 128 + pos_of


def host_prep(cfg: Cfg, h, srl_emb, src, dst, W_fc, W_feat, W_attn):
    H, D = cfg.H, cfg.D

    a = np.asarray(W_attn, F32)[0]
    a_src, a_feat = a[:D], a[2 * D:3 * D]
    W_fc = np.asarray(W_fc, F32)
    Wf_eff = (np.asarray(W_feat, F32).reshape(H, D, cfg.FEAT)
              * a_feat[None, :, None]).sum(1)
    Wz_eff = (W_fc.reshape(H, D, cfg.IN_DIM) * a_src[None, :, None]).sum(1)

    # d-major column order for z: col j <-> (h=j%8, d=j//8)
    perm = np.array([(j % H) * D + j // H for j in range(cfg.HD)], np.int64)
    Wfull = np.concatenate([W_fc.T[:, perm], Wz_eff.T], axis=1)  # [IN_DIM, HDE]
    WfcT_r = np.ascontiguousarray(
        Wfull.reshape(cfg.KC, 128, cfg.HDE).transpose(1, 0, 2)).astype(BF16)
    WfeT_r = np.ascontiguousarray(Wf_eff.T).astype(BF16)
    IotaM = np.tile(np.arange(128, dtype=F32)[None, :], (128, 1)).astype(BF16)

    h_bf = np.zeros((cfg.NA_PAD, cfg.IN_DIM), BF16)
    h_bf[:cfg.NA] = np.asarray(h, F32).astype(BF16)
    srl_bf = np.asarray(srl_emb, F32).astype(BF16)

    src = np.asarray(src, np.int64)
    dst = np.asarray(dst, np.int64)
    pair_of_edge = src // cfg.PAIR_ROWS          # src in padded node space
    slot_of_dst = _greedy_windows(cfg, dst, pair_of_edge)
    win_of_edge = slot_of_dst[dst] // 128

    # core assignment: within (pair, window), alternate between pair cores
    order = np.lexsort((src, win_of_edge, pair_of_edge))
    e_s = order
    pair_s = pair_of_edge[e_s]
    win_s = win_of_edge[e_s]
    key = pair_s * cfg.NWIN + win_s
    # rank within each (pair, window) group
    grp_start = np.r_[True, key[1:] != key[:-1]]
    gidx = np.arange(len(e_s)) - np.maximum.accumulate(
        np.where(grp_start, np.arange(len(e_s)), 0))
    core_s = pair_s * 2 + (gidx % 2)

    in_maps = []
    for c in range(cfg.NCORES):
        sel = core_s == c
        e_c = e_s[sel]                     # sorted by (window, src)
        win_c = win_s[sel]
        cnt = np.bincount(win_c, minlength=cfg.NWIN)
        assert cnt.max() <= cfg.C_WIN, f"C_WIN too small: {cnt.max()}"

        idx = np.zeros(cfg.SLOTS, np.int16)
        dstrel_v = np.full(cfg.SLOTS, -1.0, F32)
        srl_rows = np.zeros((cfg.SLOTS, cfg.FEAT), BF16)
        pos = win_c * cfg.C_WIN + (
            np.arange(len(e_c)) - np.r_[0, np.cumsum(cnt)][win_c])
        row = (src[e_c] - (c // 2) * cfg.PAIR_ROWS).astype(np.int16)
        idx[pos] = row
        dstrel_v[pos] = (slot_of_dst[dst[e_c]] - win_c * 128).astype(F32)
        srl_rows[pos] = srl_bf[e_c]
        # dummy pads: repeat a valid row (idx stays 0 where no edge before;
        # fill window pads with the window's first real row for locality)
        for w in range(cfg.NWIN):
            if cnt[w] < cfg.C_WIN:
                fill = row[np.searchsorted(win_c, w)] if cnt[w] > 0 else 0
                idx[w * cfg.C_WIN + cnt[w]:(w + 1) * cfg.C_WIN] = fill

        srlT_c = np.ascontiguousarray(srl_rows.T)

        # dstrel blocks: per (window, tile-in-window) columns
        dstrel_blk = np.full((128, cfg.NBLK), -1.0, F32)
        slots_v = dstrel_v.reshape(cfg.NTILES, 128).T   # [128, tile]
        for w in range(cfg.NWIN):
            t0, t1 = cfg.win_t0[w], cfg.win_t1[w]
            b0 = int(cfg.win_blk0[w])
            base_shift = np.zeros(t1 - t0, F32)
            # dstrel_v holds slot - win*128 for the edge's own window; for a
            # straddle tile the neighbor window's edges carry their own
            # offset.  Rebase everything to window w:
            for j, t in enumerate(range(t0, t1)):
                col = slots_v[:, t].copy()
                # which window does each slot position belong to?
                slot_ids = t * 128 + np.arange(128)
                w_of_slot = slot_ids // cfg.C_WIN
                valid = col >= 0
                rb = col + (w_of_slot - w) * 128.0
                rb[~valid] = -1.0
                dstrel_blk[:, b0 + j] = rb
        dstrel_c = dstrel_blk.astype(BF16)

        def wrap_idx(arr):  # [SLOTS] -> [128, SLOTS//16]
            wr = arr.reshape(cfg.SLOTS // 16, 16).T
            return np.ascontiguousarray(np.tile(wr, (8, 1)))

        hsl = h_bf[c * cfg.NSH:(c + 1) * cfg.NSH]
        hT_c = np.ascontiguousarray(
            hsl.T.reshape(cfg.KC, 128, cfg.NSH).transpose(1, 0, 2))

        in_maps.append({
            "hT": hT_c,
            "WfcT": WfcT_r,
            "WfeT": WfeT_r,
            "IotaM": IotaM,
            "srlT": srlT_c,
            "dstrel": dstrel_c,
            "idxT": wrap_idx(idx),
        })
    return in_maps, slot_of_dst


def required_c_win(cfg: Cfg, src, dst):
    src = np.asarray(src, np.int64)
    dst = np.asarray(dst, np.int64)
    pair_of_edge = src // cfg.PAIR_ROWS
    slot_of_dst = _greedy_windows(cfg, dst, pair_of_edge)
    win_of_edge = slot_of_dst[dst] // 128
    # worst core count: ceil(pair-window count / 2)
    key = pair_of_edge * cfg.NWIN + win_of_edge
    counts = np.bincount(key, minlength=cfg.NPAIRS * cfg.NWIN)
    need = int(-(-counts.max() // 2))
    return -(-need // 16) * 16


# --------------------------------------------------------------------------
# entry point
# --------------------------------------------------------------------------

_CACHE = {}


def _get_graph(cfg: Cfg):
    key = (cfg.NCORES, cfg.NA_PAD, cfg.NP, cfg.C_WIN, cfg.CHUNK, cfg.SP)
    if key not in _CACHE:
        _CACHE[key] = build_graph(cfg)
    return _CACHE[key]


def kernel(h, srl_emb, src, dst, W_fc, W_feat, W_attn, _trace=False,
           _tmpdir=None):
    src = np.asarray(src)
    dst = np.asarray(dst)
    cfg = Cfg()
    need = required_c_win(cfg, src, dst)
    if need != cfg.C_WIN:
        cfg = Cfg(c_win=need)
    nc = _get_graph(cfg)
    in_maps, slot_of_dst = host_prep(
        cfg, np.asarray(h), np.asarray(srl_emb), src, dst,
        np.asarray(W_fc), np.asarray(W_feat), np.asarray(W_attn))
    res = run_bass_kernel_spmd(nc, in_maps, core_ids=list(range(cfg.NCORES)),
                               trace=_trace, tmpdir=_tmpdir)
    # reassemble: segment s, core c shard covers slots s*2048 + c*256 + [0,256)
    out_slots = np.empty((cfg.SLOT_ROWS, cfg.H, cfg.D), F32)
    NSEG = 2
    SPS = cfg.SLOT_ROWS // NSEG
    SHR = SPS // cfg.NCORES
    for c in range(cfg.NCORES):
        shard = np.asarray(res.results[c]["out"], F32)  # [1280, 256] d-major
        shard = shard.reshape(NSEG, SHR, cfg.D, cfg.H).transpose(0, 1, 3, 2)
        for s in range(NSEG):
            out_slots[s * SPS + c * SHR:s * SPS + (c + 1) * SHR] = shard[s]
    out = out_slots[slot_of_dst]
    if _trace:
        kernel._last_results = res
    return out


# revision 40
# speedup vs baseline: 1.3433x; 1.0280x over previous
"""Trainium2 distributed Bass kernel for the APGAT layer (gnn_message_passing).

v2 strategy (8 NeuronCores, 4 HBM pairs):
  - Nodes are sharded over cores (6272/core); cores (2k, 2k+1) share an HBM
    domain, so each pair holds a PAIR-LOCAL z table of 12544 rows in Shared
    DRAM (written with a rank-dependent dynamic-offset DMA; a tiny pair
    AllReduce acts as the readiness barrier).  12544 < 32768 so gather
    indices fit int16 with a SINGLE table - no A/B split.
  - Edges are assigned to the pair that owns their src node (gathers are
    always pair-local; no z AllGather at all), split evenly between the two
    cores of the pair.  Each core processes ~50k edges spanning ALL dst
    nodes; per-dst partial sums are combined at the end with two pipelined
    8-core ReduceScatters over the 10240 dst slots.
  - dst nodes are relabeled on the host (greedy, degree-balanced) into 80
    windows of 125 nodes so every (core, window) has <= C_WIN edges.
    Edge stream = 80 windows x C_WIN slots, padded with dummy (valid) idx
    rows that dstrel = -1 masks out of the one-hot.
  - Per 1792-edge chunk: SWDGE dma_gather of [z | s_src] rows (768 B),
    s_feat via PE (srl stationary), scores -> exp, msg = ex (x) z in the
    DVE 2x mode (d-major layout).  Per window: one-hot is_equal + PE
    segment-sum matmuls into PSUM, evicted bf16 to an SBUF accumulator.
  - Softmax max-subtraction is skipped (scores are O(7); validated vs the
    reference, gate is 2e-2).
"""

import sys

sys.path.insert(0, "/opt/trn_rl_repo")

import numpy as np
import ml_dtypes

import concourse.bass as bass
import concourse.bacc as bacc
import concourse.mybir as mybir
import concourse.tile as tile
from concourse.bass import ds
from concourse.tile import add_dep_helper
from concourse.bass_utils import run_bass_kernel_spmd

BF16 = ml_dtypes.bfloat16
F8 = ml_dtypes.float8_e4m3fn
F32 = np.float32
AF = mybir.ActivationFunctionType
ALU = mybir.AluOpType

N_SWDGE_Q = 8


class Cfg:
    def __init__(self, ncores=8, na=50000, np_=10000, e=400000, in_dim=512,
                 feat=128, h=8, d=32, c_win=640, chunk=640, single_packet=True):
        self.NCORES = ncores
        self.NPAIRS = ncores // 2
        self.NA = na
        self.NP = np_
        self.E = e
        self.IN_DIM = in_dim
        self.KC = in_dim // 128
        self.FEAT = feat
        self.H = h
        self.D = d
        self.HD = h * d                    # 256
        self.HDE = self.HD + h             # 264: [z | s_src]
        self.RW = 384                      # bf16 row width -> 768 B rows
        nsh = -(-na // ncores)
        self.NSH = -(-nsh // 128) * 128    # nodes per core (6272)
        self.NA_PAD = self.NSH * ncores
        self.NT = self.NSH // 128          # node tiles per core (49)
        self.PAIR_ROWS = 2 * self.NSH      # 12544 (< 32768: int16 idx)
        assert self.PAIR_ROWS <= 32768
        # dst windows: 80 windows x 125 nodes (128 slots each)
        self.NWIN = 80
        self.NODES_PER_WIN = np_ // self.NWIN    # 125
        self.SLOT_ROWS = self.NWIN * 128         # 10240 dst slots
        self.WIN_PER_CORE = self.NWIN // ncores  # 10
        # per (core, window) edge capacity
        assert c_win % 16 == 0
        self.C_WIN = c_win
        self.CHUNK = chunk                 # gather chunk (multiple of 128)
        assert chunk % 128 == 0
        self.SP = single_packet
        slots = self.NWIN * c_win
        self.NCHUNK = -(-slots // chunk)
        self.SLOTS = self.NCHUNK * chunk   # padded stream length
        self.TPC = chunk // 128            # tiles per chunk
        self.NTILES = self.SLOTS // 128
        # per-window tile spans (static given C_WIN)
        self.win_t0 = [(c_win * w) // 128 for w in range(self.NWIN)]
        self.win_t1 = [-(-(c_win * (w + 1)) // 128) for w in range(self.NWIN)]
        self.win_blk0 = np.cumsum([0] + [t1 - t0 for t0, t1 in
                                         zip(self.win_t0, self.win_t1)])
        self.NBLK = int(self.win_blk0[-1])


def build_graph(cfg: Cfg):
    nc = bacc.Bacc("TRN2", target_bir_lowering=False, debug=False,
                   num_devices=cfg.NCORES, num_swdge_queues=N_SWDGE_Q)
    bf = mybir.dt.bfloat16
    f8 = mybir.dt.float8e4
    f32 = mybir.dt.float32
    i16 = mybir.dt.int16
    u8 = mybir.dt.uint8

    # ---- kernel I/O ----
    hT = nc.dram_tensor("hT", [128, cfg.KC, cfg.NSH], bf, kind="ExternalInput")
    WfcT = nc.dram_tensor("WfcT", [128, cfg.KC, cfg.HDE], bf, kind="ExternalInput")
    WfeT = nc.dram_tensor("WfeT", [cfg.FEAT, cfg.H], bf, kind="ExternalInput")
    IotaM = nc.dram_tensor("IotaM", [128, 128], bf, kind="ExternalInput")
    srlT = nc.dram_tensor("srlT", [cfg.FEAT, cfg.SLOTS], bf, kind="ExternalInput")
    dstrel = nc.dram_tensor("dstrel", [128, cfg.NBLK], bf, kind="ExternalInput")
    idxT = nc.dram_tensor("idxT", [128, cfg.SLOTS // 16], i16, kind="ExternalInput")
    out_ext = nc.dram_tensor("out", [2 * cfg.WIN_PER_CORE // 2 * 128, cfg.HD],
                             f32, kind="ExternalOutput")   # [1280, 256]

    pair_groups = [[2 * p, 2 * p + 1] for p in range(cfg.NPAIRS)]
    all_group = [list(range(cfg.NCORES))]
    NSEG = 2                               # pipelined ReduceScatter halves
    WPS = cfg.NWIN // NSEG                 # windows per segment (16)
    SPS = WPS * 128                        # acc rows per segment (2048)
    SHR = SPS // cfg.NCORES                # shard rows per core (256)
    SHT = SHR // 128                       # shard row tiles (2)

    with tile.TileContext(nc) as tc:
        with (
            tc.tile_pool(name="dram", bufs=1, space="DRAM") as dram,
            tc.tile_pool(name="consts", bufs=1) as consts,
            tc.tile_pool(name="psum_sf", bufs=2, space="PSUM") as psum_sf,
            tc.tile_pool(name="psum_acc", bufs=2, space="PSUM") as psum_acc,
            tc.tile_pool(name="srl", bufs=4) as srl_pool,
            tc.tile_pool(name="zg", bufs=7) as zg_pool,
            tc.tile_pool(name="msg", bufs=6) as msg_pool,
            tc.tile_pool(name="oh", bufs=3) as oh_pool,
            tc.tile_pool(name="small", bufs=3) as small,
            tc.tile_pool(name="accsb", bufs=1) as accsb,
            tc.tile_pool(name="fin", bufs=2) as fin,
        ):
            # ---- shared/DRAM scratch ----
            z_pair = dram.tile([cfg.PAIR_ROWS, cfg.RW], bf, addr_space="Shared")
            bar_in = dram.tile([1, 16], bf)
            bar_out = dram.tile([1, 16], bf)
            acc_dram = [dram.tile([SPS, cfg.HDE], bf, name=f"accd{i}")
                        for i in range(NSEG)]
            rs_out = [dram.tile([SHR, cfg.HDE], bf, name=f"rs{i}")
                      for i in range(NSEG)]

            # ---- constants ----
            wfe_sb = consts.tile([cfg.FEAT, cfg.H], bf)
            nc.sync.dma_start(wfe_sb[:], WfeT[:])
            iota_sb = consts.tile([128, 128], bf)
            nc.sync.dma_start(iota_sb[:], IotaM[:])
            idx_sb = consts.tile([128, cfg.SLOTS // 16], i16)
            nc.sync.dma_start(idx_sb[:], idxT[:])
            dst_sb = consts.tile([128, cfg.NBLK], bf)
            nc.sync.dma_start(dst_sb[:], dstrel[:])

            # rank within the pair (0/1) for the z-table write offset
            rank1 = nc.sync.partition_id() % 2
            row_off = rank1 * cfg.NSH

            # ---- phase A: z = [h @ Wfc | s_src] -> pair-shared table ----
            with (
                tc.tile_pool(name="zph_h", bufs=2) as zph_h,
                tc.tile_pool(name="zph_w", bufs=1) as zph_w,
                tc.tile_pool(name="zph_s", bufs=1) as zph_s,
                tc.tile_pool(name="psum_z", bufs=4, space="PSUM") as psum_z,
            ):
                wfc_sb = zph_w.tile([128, cfg.KC, cfg.HDE], bf)
                nc.sync.dma_start(wfc_sb[:], WfcT[:])
                zstage = zph_s.tile([128, cfg.NT, cfg.HDE], bf)
                bounds = [0, 13, 25, 37, cfg.NT]
                for t0, t1 in zip(bounds[:-1], bounds[1:]):
                    nt = t1 - t0
                    hT_sb = zph_h.tile([128, cfg.KC, 13 * 128], bf,
                                       name=f"hT{t0}", tag="hT")
                    nc.sync.dma_start(hT_sb[:, :, 0:nt * 128],
                                      hT[:, :, t0 * 128:t1 * 128])
                    for i in range(nt):
                        pz = psum_z.tile([128, cfg.HDE], f32,
                                         name=f"pz{t0 + i}", tag="pz")
                        for c in range(cfg.KC):
                            nc.tensor.matmul(
                                pz[:],
                                hT_sb[:, c, i * 128:(i + 1) * 128],
                                wfc_sb[:, c, :],
                                start=(c == 0), stop=(c == cfg.KC - 1),
                            )
                        nc.scalar.copy(zstage[:, t0 + i, :], pz[:])
                z_write = nc.sync.dma_start(
                    z_pair[:][ds(row_off, cfg.NSH), 0:cfg.HDE]
                        .rearrange("(t p) r -> p t r", p=128),
                    zstage[:])

            # barrier: pair AllReduce; completes only after both pair cores
            # have finished their z-table writes
            nc.sync.dma_start(bar_in[:], iota_sb[0:1, 0:16])
            bar = nc.gpsimd.collective_compute(
                "AllReduce", ALU.add, ins=[bar_in[:].opt()],
                outs=[bar_out[:].opt()], replica_groups=pair_groups)
            add_dep_helper(bar.ins, z_write.ins, reason="barrier after z write")

            # ---- phase B ----
            # windows grouped by the chunk that completes them
            win_by_chunk = {}
            for w in range(cfg.NWIN):
                lc = (cfg.win_t1[w] - 1) // cfg.TPC
                win_by_chunk.setdefault(lc, []).append(w)

            msg_tiles = {}    # global tile idx -> (msg tile, local idx)
            evicts = []
            rs_pending = []
            acc_sb = [accsb.tile([128, WPS, cfg.HDE], bf, name=f"acc{i}")
                      for i in range(NSEG)]

            # prefetch: first srl chunks + one-hot blocks run during phase A
            srl_pre = {}
            for c in range(3):
                srl_sb = srl_pool.tile([cfg.FEAT, cfg.CHUNK], bf,
                                       name=f"srl{c}", tag="srl")
                nc.scalar.dma_start(
                    srl_sb[:], srlT[:, c * cfg.CHUNK:(c + 1) * cfg.CHUNK])
                srl_pre[c] = srl_sb

            oh_pre = {}

            def build_oh(w):
                t0, t1 = cfg.win_t0[w], cfg.win_t1[w]
                ntw = t1 - t0
                blk0 = int(cfg.win_blk0[w])
                oh = oh_pool.tile([128, ntw, 128], bf, name=f"oh{w}", tag="oh")
                nc.vector.tensor_tensor(
                    oh[:],
                    iota_sb[:].unsqueeze(1).broadcast_to([128, ntw, 128]),
                    dst_sb[:, blk0:blk0 + ntw]
                        .unsqueeze(2).broadcast_to([128, ntw, 128]),
                    ALU.is_equal,
                )
                return oh

            for w in range(3):
                oh_pre[w] = build_oh(w)

            def emit_window(w):
                t0, t1 = cfg.win_t0[w], cfg.win_t1[w]
                ntw = t1 - t0
                oh = oh_pre.pop(w, None)
                if oh is None:
                    oh = build_oh(w)
                pacc = psum_acc.tile([128, cfg.HDE], f32,
                                     name=f"pacc{w}", tag="pacc")
                for j, t in enumerate(range(t0, t1)):
                    mt, li = msg_tiles[t]
                    nc.tensor.matmul(
                        pacc[:],
                        oh[:, j, :],
                        mt[:, li, :],
                        start=(j == 0), stop=(j == ntw - 1),
                    )
                seg, wl = w // WPS, w % WPS
                cp = nc.scalar.copy(acc_sb[seg][:, wl, :], pacc[:])
                evicts.append(cp.ins)

            def emit_rs(seg):
                nc.sync.dma_start(
                    acc_dram[seg][:]
                        .rearrange("(w p) c -> p w c", p=128),
                    acc_sb[seg][:])
                nc.gpsimd.collective_compute(
                    "ReduceScatter", ALU.add,
                    ins=[acc_dram[seg][:].opt()],
                    outs=[rs_out[seg][:].opt()],
                    replica_groups=all_group)

            for c in range(cfg.NCHUNK):
                zg = zg_pool.tile([128, cfg.TPC, cfg.RW], bf,
                                  name=f"zg{c}", tag="zg")
                g = nc.gpsimd.dma_gather(
                    zg[:], z_pair[:],
                    idx_sb[:, c * (cfg.CHUNK // 16):(c + 1) * (cfg.CHUNK // 16)],
                    cfg.CHUNK, cfg.CHUNK, cfg.RW,
                    single_packet=cfg.SP,
                    queue_num=c % N_SWDGE_Q,
                )
                add_dep_helper(g.ins, bar.ins, reason="gather after barrier")

                srl_sb = srl_pre.pop(c, None)
                if srl_sb is None:
                    srl_sb = srl_pool.tile([cfg.FEAT, cfg.CHUNK], bf,
                                           name=f"srl{c}", tag="srl")
                    nc.scalar.dma_start(
                        srl_sb[:], srlT[:, c * cfg.CHUNK:(c + 1) * cfg.CHUNK])

                # scores: copy s_src into PSUM, matmul-accumulate s_feat on
                # top (start=False), then lrelu+exp on the Act engine
                psf = psum_sf.tile([128, cfg.TPC, cfg.H], f32,
                                   name=f"psf{c}", tag="psf")
                nc.scalar.copy(psf[:], zg[:, :, cfg.HD:cfg.HDE])
                for t in range(cfg.TPC):
                    nc.tensor.matmul(
                        psf[:, t, :],
                        srl_sb[:, t * 128:(t + 1) * 128],
                        wfe_sb[:],
                        start=False, stop=True,
                    )
                sfs = small.tile([128, cfg.TPC, cfg.H], f32,
                                 name=f"sfs{c}", tag="sfs")
                nc.scalar.copy(sfs[:], psf[:])
                lr = small.tile([128, cfg.TPC, cfg.H], f32,
                                name=f"lr{c}", tag="lr")
                nc.vector.scalar_tensor_tensor(
                    lr[:], sfs[:], 0.01, sfs[:], ALU.mult, ALU.max)

                msg = msg_pool.tile([128, cfg.TPC, cfg.HDE], bf,
                                    name=f"msg{c}", tag="msg")
                nc.scalar.activation(msg[:, :, cfg.HD:], lr[:], AF.Exp)
                nc.vector.tensor_tensor(
                    msg[:, :, 0:cfg.HD].rearrange("p t (d h) -> p t d h", h=cfg.H),
                    zg[:, :, 0:cfg.HD].rearrange("p t (d h) -> p t d h", h=cfg.H),
                    msg[:, :, cfg.HD:].unsqueeze(2)
                        .broadcast_to([128, cfg.TPC, cfg.D, cfg.H]),
                    ALU.mult,
                )
                for t in range(cfg.TPC):
                    msg_tiles[c * cfg.TPC + t] = (msg, t)

                for w in win_by_chunk.get(c, []):
                    emit_window(w)
                    if w % WPS == WPS - 1:
                        rs_pending.append((c + 2, w // WPS))
                while rs_pending and rs_pending[0][0] <= c:
                    emit_rs(rs_pending.pop(0)[1])
            while rs_pending:
                emit_rs(rs_pending.pop(0)[1])

            # ---- finalize: out = msg_tot / max(den, eps) per RS shard ----
            for seg in range(NSEG):
                tot = fin.tile([128, SHT, cfg.HDE], bf,
                               name=f"tot{seg}", tag="tot")
                tl = nc.sync.dma_start(
                    tot[:],
                    rs_out[seg][:].rearrange("(w p) c -> p w c", p=128))
                add_dep_helper(tl.ins, evicts[-1],
                               reason="finalize after last evict")
                den = fin.tile([128, SHT, cfg.H], f32,
                               name=f"den{seg}", tag="den")
                nc.scalar.activation(den[:], tot[:, :, cfg.HD:cfg.HDE],
                                     AF.Copy, bias=1e-9)
                rec = fin.tile([128, SHT, cfg.H], f32,
                               name=f"rec{seg}", tag="rec")
                nc.vector.reciprocal(rec[:], den[:])
                ow = fin.tile([128, SHT, cfg.HD], f32,
                              name=f"ow{seg}", tag="ow")
                nc.vector.tensor_tensor(
                    ow[:].rearrange("p w (d h) -> p w d h", h=cfg.H),
                    tot[:, :, 0:cfg.HD].rearrange("p w (d h) -> p w d h", h=cfg.H),
                    rec[:].unsqueeze(2)
                        .broadcast_to([128, SHT, cfg.D, cfg.H]),
                    ALU.mult,
                )
                nc.sync.dma_start(
                    out_ext[seg * SHR:(seg + 1) * SHR, :]
                        .rearrange("(w p) c -> p w c", p=128),
                    ow[:])

    nc.compile()
    return nc


# --------------------------------------------------------------------------
# host-side preprocessing
# --------------------------------------------------------------------------

def _greedy_windows(cfg: Cfg, dst, pair_of_edge):
    """Assign dst nodes to 80 windows of 125, balancing per-pair edge load.
    Returns slot_of_dst [NP] (window*128 + position)."""
    NW = cfg.NWIN
    # per (dst, pair) degree
    deg = np.zeros((cfg.NP, cfg.NPAIRS), np.int64)
    np.add.at(deg, (dst, pair_of_edge), 1)
    tot = deg.sum(1)
    order = np.argsort(-tot, kind="stable")
    load = np.zeros((NW, cfg.NPAIRS), np.int64)
    count = np.zeros(NW, np.int64)
    win_of = np.empty(cfg.NP, np.int64)
    pos_of = np.empty(cfg.NP, np.int64)
    for d in order:
        cand = load + deg[d][None, :]
        score = cand.max(1) * 1000 + cand.sum(1)
        score[count >= cfg.NODES_PER_WIN] = np.iinfo(np.int64).max
        w = int(np.argmin(score))
        win_of[d] = w
        pos_of[d] = count[w]
        count[w] += 1
        load[w] += deg[d]
    assert (count == cfg.NODES_PER_WIN).all()
    return win_of * 128 + pos_of


def host_prep(cfg: Cfg, h, srl_emb, src, dst, W_fc, W_feat, W_attn):
    H, D = cfg.H, cfg.D

    a = np.asarray(W_attn, F32)[0]
    a_src, a_feat = a[:D], a[2 * D:3 * D]
    W_fc = np.asarray(W_fc, F32)
    Wf_eff = (np.asarray(W_feat, F32).reshape(H, D, cfg.FEAT)
              * a_feat[None, :, None]).sum(1)
    Wz_eff = (W_fc.reshape(H, D, cfg.IN_DIM) * a_src[None, :, None]).sum(1)

    # d-major column order for z: col j <-> (h=j%8, d=j//8)
    perm = np.array([(j % H) * D + j // H for j in range(cfg.HD)], np.int64)
    Wfull = np.concatenate([W_fc.T[:, perm], Wz_eff.T], axis=1)  # [IN_DIM, HDE]
    WfcT_r = np.ascontiguousarray(
        Wfull.reshape(cfg.KC, 128, cfg.HDE).transpose(1, 0, 2)).astype(BF16)
    WfeT_r = np.ascontiguousarray(Wf_eff.T).astype(BF16)
    IotaM = np.tile(np.arange(128, dtype=F32)[None, :], (128, 1)).astype(BF16)

    h_bf = np.zeros((cfg.NA_PAD, cfg.IN_DIM), BF16)
    h_bf[:cfg.NA] = np.asarray(h, F32).astype(BF16)
    srl_bf = np.asarray(srl_emb, F32).astype(BF16)

    src = np.asarray(src, np.int64)
    dst = np.asarray(dst, np.int64)
    pair_of_edge = src // cfg.PAIR_ROWS          # src in padded node space
    slot_of_dst = _greedy_windows(cfg, dst, pair_of_edge)
    win_of_edge = slot_of_dst[dst] // 128

    # core assignment: within (pair, window), alternate between pair cores
    order = np.lexsort((src, win_of_edge, pair_of_edge))
    e_s = order
    pair_s = pair_of_edge[e_s]
    win_s = win_of_edge[e_s]
    key = pair_s * cfg.NWIN + win_s
    # rank within each (pair, window) group
    grp_start = np.r_[True, key[1:] != key[:-1]]
    gidx = np.arange(len(e_s)) - np.maximum.accumulate(
        np.where(grp_start, np.arange(len(e_s)), 0))
    core_s = pair_s * 2 + (gidx % 2)

    in_maps = []
    for c in range(cfg.NCORES):
        sel = core_s == c
        e_c = e_s[sel]                     # sorted by (window, src)
        win_c = win_s[sel]
        cnt = np.bincount(win_c, minlength=cfg.NWIN)
        assert cnt.max() <= cfg.C_WIN, f"C_WIN too small: {cnt.max()}"

        idx = np.zeros(cfg.SLOTS, np.int16)
        dstrel_v = np.full(cfg.SLOTS, -1.0, F32)
        srl_rows = np.zeros((cfg.SLOTS, cfg.FEAT), BF16)
        pos = win_c * cfg.C_WIN + (
            np.arange(len(e_c)) - np.r_[0, np.cumsum(cnt)][win_c])
        row = (src[e_c] - (c // 2) * cfg.PAIR_ROWS).astype(np.int16)
        idx[pos] = row
        dstrel_v[pos] = (slot_of_dst[dst[e_c]] - win_c * 128).astype(F32)
        srl_rows[pos] = srl_bf[e_c]
        # dummy pads: repeat a valid row (idx stays 0 where no edge before;
        # fill window pads with the window's first real row for locality)
        for w in range(cfg.NWIN):
            if cnt[w] < cfg.C_WIN:
                fill = row[np.searchsorted(win_c, w)] if cnt[w] > 0 else 0
                idx[w * cfg.C_WIN + cnt[w]:(w + 1) * cfg.C_WIN] = fill

        srlT_c = np.ascontiguousarray(srl_rows.T)

        # dstrel blocks: per (window, tile-in-window) columns
        dstrel_blk = np.full((128, cfg.NBLK), -1.0, F32)
        slots_v = dstrel_v.reshape(cfg.NTILES, 128).T   # [128, tile]
        for w in range(cfg.NWIN):
            t0, t1 = cfg.win_t0[w], cfg.win_t1[w]
            b0 = int(cfg.win_blk0[w])
            base_shift = np.zeros(t1 - t0, F32)
            # dstrel_v holds slot - win*128 for the edge's own window; for a
            # straddle tile the neighbor window's edges carry their own
            # offset.  Rebase everything to window w:
            for j, t in enumerate(range(t0, t1)):
                col = slots_v[:, t].copy()
                # which window does each slot position belong to?
                slot_ids = t * 128 + np.arange(128)
                w_of_slot = slot_ids // cfg.C_WIN
                valid = col >= 0
                rb = col + (w_of_slot - w) * 128.0
                rb[~valid] = -1.0
                dstrel_blk[:, b0 + j] = rb
        dstrel_c = dstrel_blk.astype(BF16)

        def wrap_idx(arr):  # [SLOTS] -> [128, SLOTS//16]
            wr = arr.reshape(cfg.SLOTS // 16, 16).T
            return np.ascontiguousarray(np.tile(wr, (8, 1)))

        hsl = h_bf[c * cfg.NSH:(c + 1) * cfg.NSH]
        hT_c = np.ascontiguousarray(
            hsl.T.reshape(cfg.KC, 128, cfg.NSH).transpose(1, 0, 2))

        in_maps.append({
            "hT": hT_c,
            "WfcT": WfcT_r,
            "WfeT": WfeT_r,
            "IotaM": IotaM,
            "srlT": srlT_c,
            "dstrel": dstrel_c,
            "idxT": wrap_idx(idx),
        })
    return in_maps, slot_of_dst


def required_c_win(cfg: Cfg, src, dst):
    src = np.asarray(src, np.int64)
    dst = np.asarray(dst, np.int64)
    pair_of_edge = src // cfg.PAIR_ROWS
    slot_of_dst = _greedy_windows(cfg, dst, pair_of_edge)
    win_of_edge = slot_of_dst[dst] // 128
    # worst core count: ceil(pair-window count / 2)
    key = pair_of_edge * cfg.NWIN + win_of_edge
    counts = np.bincount(key, minlength=cfg.NPAIRS * cfg.NWIN)
    need = int(-(-counts.max() // 2))
    return -(-need // 16) * 16


# --------------------------------------------------------------------------
# entry point
# --------------------------------------------------------------------------

_CACHE = {}


def _get_graph(cfg: Cfg):
    key = (cfg.NCORES, cfg.NA_PAD, cfg.NP, cfg.C_WIN, cfg.CHUNK, cfg.SP)
    if key not in _CACHE:
        _CACHE[key] = build_graph(cfg)
    return _CACHE[key]


def kernel(h, srl_emb, src, dst, W_fc, W_feat, W_attn, _trace=False,
           _tmpdir=None):
    src = np.asarray(src)
    dst = np.asarray(dst)
    cfg = Cfg()
    need = required_c_win(cfg, src, dst)
    if need != cfg.C_WIN:
        cfg = Cfg(c_win=need)
    nc = _get_graph(cfg)
    in_maps, slot_of_dst = host_prep(
        cfg, np.asarray(h), np.asarray(srl_emb), src, dst,
        np.asarray(W_fc), np.asarray(W_feat), np.asarray(W_attn))
    res = run_bass_kernel_spmd(nc, in_maps, core_ids=list(range(cfg.NCORES)),
                               trace=_trace, tmpdir=_tmpdir)
    # reassemble: segment s, core c shard covers slots s*2048 + c*256 + [0,256)
    out_slots = np.empty((cfg.SLOT_ROWS, cfg.H, cfg.D), F32)
    NSEG = 2
    SPS = cfg.SLOT_ROWS // NSEG
    SHR = SPS // cfg.NCORES
    for c in range(cfg.NCORES):
        shard = np.asarray(res.results[c]["out"], F32)  # [1280, 256] d-major
        shard = shard.reshape(NSEG, SHR, cfg.D, cfg.H).transpose(0, 1, 3, 2)
        for s in range(NSEG):
            out_slots[s * SPS + c * SHR:s * SPS + (c + 1) * SHR] = shard[s]
    out = out_slots[slot_of_dst]
    if _trace:
        kernel._last_results = res
    return out
